# revision 19
# baseline (speedup 1.0000x reference)
"""Trainium2 Bass kernel for nn_Encoder (GNN message passing), 8-core SPMD.

Strategy (edge-parallel):
  - Shard E edges across 8 cores; each core sorts its edges by dst node and
    processes them in 128-node "windows" so scatter-add becomes PSUM-matmul
    accumulation against on-device one-hot selection matrices.
  - Gather n[dst] is fused into the message MLP layer 1 by precomputing
    P = n @ w1_node per node; the gather matmul accumulates P[dst] directly
    into the L1 PSUM tile.
  - conv1 node sums accumulate transposed ([H, node]) and are AllReduced;
    conv2 sums accumulate natural ([node, H]) and are ReduceScattered to
    feed the input-dim-sharded graph head (graph_w in bf16, streamed).
  - e / e2 / node MLPs run in float32r; conv machinery runs in bf16 with
    fp32 PSUM accumulation.
"""

import math
import numpy as np
import ml_dtypes

import concourse.bacc as bacc
import concourse.bass as bass
import concourse.mybir as mybir
import concourse.tile as tile
from concourse.bass_utils import run_bass_kernel_spmd

F32 = mybir.dt.float32
F32R = mybir.dt.float32r
BF16 = mybir.dt.bfloat16

# problem constants (full size)
N = 10000
E = 320000
H = 128
NCORES = 8
IN_NODE = 4
IN_EDGE = 3
SLOPE = 0.1
WIN = 128          # nodes per scatter window
TS = 512           # edge slots per streaming tile
H1 = 64            # hidden dim of the 2-layer MLPs
H1A = H1 + 1       # augmented with ones-row for bias folding


# ---------------------------------------------------------------- host prep

def build_schedule(dst, n_nodes, e_loc, ncores):
    """Identical-across-cores chunk schedule, per-core slot assignments.

    Returns (C, win_of_chunk, chunk0_of_win, EP, slots) where slots[c] is an
    int64 array [EP] holding the LOCAL edge index for each slot or -1.
    """
    nw = math.ceil(n_nodes / WIN)
    cnt = np.zeros((ncores, nw), np.int64)
    orders = []
    for c in range(ncores):
        d = dst[c * e_loc:(c + 1) * e_loc]
        order = np.argsort(d, kind="stable")
        orders.append(order)
        cnt[c] = np.bincount(d // WIN, minlength=nw)
    C = np.maximum(1, np.ceil(cnt.max(axis=0) / 128).astype(np.int64))
    # pad so that total chunks is a multiple of TS//128 (tiles are uniform)
    per_tile = TS // 128
    pad = (-int(C.sum())) % per_tile
    C[nw - 1] += pad
    nch = int(C.sum())
    ep = nch * 128
    chunk0 = np.zeros(nw, np.int64)
    chunk0[1:] = np.cumsum(C)[:-1]
    win_of_chunk = np.repeat(np.arange(nw), C)
    slots = []
    for c in range(ncores):
        d = dst[c * e_loc:(c + 1) * e_loc]
        order = orders[c]
        slot = np.full(ep, -1, np.int64)
        # sorted edges of window w occupy slots starting at 128*chunk0[w]
        start_in_sorted = np.zeros(nw, np.int64)
        start_in_sorted[1:] = np.cumsum(cnt[c])[:-1]
        for w in range(nw):
            k = int(cnt[c, w])
            if k == 0:
                continue
            base = 128 * int(chunk0[w])
            slot[base:base + k] = order[start_in_sorted[w]:start_in_sorted[w] + k]
        slots.append(slot)
    return C, win_of_chunk, chunk0, ep, slots


def host_arrays(inputs, n_nodes, n_edges, ncores):
    """Build all per-core and shared device input arrays."""
    e_loc = n_edges // ncores
    nw = math.ceil(n_nodes / WIN)
    np_pad = nw * WIN
    dst = np.asarray(inputs["edge_index"][1])
    C, win_of_chunk, chunk0, ep, slots = build_schedule(dst, n_nodes, e_loc, ncores)
    nch = ep // 128
    ga = math.ceil(nch / 128)

    bf = lambda x: np.ascontiguousarray(x).astype(ml_dtypes.bfloat16)
    f32 = lambda x: np.ascontiguousarray(x, np.float32)

    shared = {}
    # node MLP (float32r path, stored f32)
    shared["nw1"] = bf(inputs["node_w1"])                 # [4,64]
    shared["nb1"] = f32(np.asarray(inputs["node_b1"])[:, None])  # [64,1]
    shared["nw2a"] = bf(np.concatenate(
        [inputs["node_w2"], inputs["node_b2"][None, :]], axis=0))  # [65,128]
    # edge MLP
    shared["ew1"] = bf(inputs["edge_w1"])                 # [3,64]
    shared["eb1"] = f32(np.asarray(inputs["edge_b1"])[:, None])
    shared["ew2a"] = bf(np.concatenate(
        [inputs["edge_w2"], inputs["edge_b2"][None, :]], axis=0))  # [65,128]
    # e2 MLP (last=True edge transform)
    shared["e2w1"] = bf(inputs["g2e_w1"])                 # [128,64]
    shared["e2b1"] = f32(np.asarray(inputs["g2e_b1"])[:, None])
    shared["e2w2a"] = bf(np.concatenate(
        [inputs["g2e_w2"], inputs["g2e_b2"][None, :]], axis=0))  # [65,128]
    # conv weights (bf16 machinery)
    for i, pre in ((1, "g1"), (2, "g2")):
        w1 = np.asarray(inputs[f"{pre}_w1"])  # [2H, 64]
        shared[f"c{i}_w1n"] = bf(w1[:H])                      # [128,64]
        shared[f"c{i}_w1e"] = bf(w1[H:])                     # [128,64] f32r
        shared[f"c{i}_b1"] = f32(np.asarray(inputs[f"{pre}_b1"])[:, None])  # [64,1]
        shared[f"c{i}_w2a"] = bf(np.concatenate(
            [inputs[f"{pre}_w2"], inputs[f"{pre}_b2"][None, :]], axis=0))  # [65,128]
    # consts
    shared["iotaFb"] = f32(np.tile(np.arange(128, dtype=np.float32), (128, 1)))
    shared["idn"] = f32(np.eye(128, dtype=np.float32))
    shared["idnb"] = bf(np.eye(128, dtype=np.float32))
    # node features, padded + transposed
    nf = np.zeros((np_pad, IN_NODE), np.float32)
    nf[:n_nodes] = inputs["node_feat"]
    shared["nfT"] = bf(nf.T)  # [4, NP]

    # per-core arrays
    per_core = []
    ef = np.asarray(inputs["edge_feat"])
    rs_rows = np_pad // ncores
    gw = np.zeros((np_pad * H, H), np.float32)
    gw[:n_nodes * H] = inputs["graph_w"]
    for c in range(ncores):
        slot = slots[c]
        valid = slot >= 0
        d = dst[c * e_loc:(c + 1) * e_loc]
        eft = np.zeros((IN_EDGE, ep), np.float32)
        eft[:, valid] = ef[c * e_loc + slot[valid]].T
        eft = bf(eft)
        dstoff = np.full(ep, -1.0, np.float32)
        dstoff[valid] = (d[slot[valid]] - WIN * win_of_chunk[
            np.arange(ep) // 128][valid]).astype(np.float32)
        dstB = dstoff.reshape(nch, 128).T.copy()       # [128, NCH]
        # graph head weight shard, shuffled so rhs tiles DMA contiguously:
        # gws[k, 128*i + h] = gw_shard[128*i + k, h]
        gshard = gw[c * rs_rows * H:(c + 1) * rs_rows * H]   # [rs_rows*H, H]
        nchunk = gshard.shape[0] // 128
        gws = bf(gshard.reshape(nchunk, 128, H).transpose(1, 0, 2).reshape(128, -1))
        per_core.append({
            "efT": eft, "dstB": dstB, "gws": gws,
            "slot_global": np.where(valid, c * e_loc + slot, -1),
        })
    meta = dict(C=C, win_of_chunk=win_of_chunk, chunk0=chunk0, ep=ep,
                nch=nch, ga=ga, nw=nw, np_pad=np_pad, e_loc=e_loc,
                rs_rows=rs_rows)
    return shared, per_core, meta


# ---------------------------------------------------------------- device prog

def build_program(meta, n_cores):
    nw, np_pad, ep, nch, ga = (meta["nw"], meta["np_pad"], meta["ep"],
                               meta["nch"], meta["ga"])
    C, win_of_chunk, chunk0 = meta["C"], meta["win_of_chunk"], meta["chunk0"]
    rs_rows = meta["rs_rows"]
    nt = ep // TS
    per_tile = TS // 128

    nc = bacc.Bacc(num_devices=n_cores)
    core_ids = list(range(n_cores))

    # ---- I/O declarations
    inp = {}
    def din(name, shape, dtype=F32):
        inp[name] = nc.declare_dram_parameter(name, list(shape), dtype, isOutput=False)
        return inp[name]

    for nm, sh in [
        ("nb1", (H1, 1)), ("eb1", (H1, 1)), ("e2b1", (H1, 1)),
        ("c1_b1", (H1, 1)), ("c2_b1", (H1, 1)),
        ("iotaFb", (128, 128)), ("idn", (128, 128)),
        ("dstB", (128, nch)),
    ]:
        din(nm, sh)
    for nm, sh in [("nw1", (IN_NODE, H1)), ("nw2a", (H1A, H)),
                   ("ew1", (IN_EDGE, H1)), ("ew2a", (H1A, H)),
                   ("e2w1", (H, H1)), ("e2w2a", (H1A, H)),
                   ("c1_w1e", (H, H1)), ("c2_w1e", (H, H1)),
                   ("c1_w1n", (H, H1)), ("c1_w2a", (H1A, H)),
                   ("c2_w1n", (H, H1)), ("c2_w2a", (H1A, H)),
                   ("idnb", (128, 128)), ("nfT", (IN_NODE, np_pad)),
                   ("efT", (IN_EDGE, ep)),
                   ("gws", (128, rs_rows * H))]:
        din(nm, sh, BF16)

    eT_out = nc.declare_dram_parameter("eT_out", [128, ep], BF16, isOutput=True)
    gpart = nc.declare_dram_parameter("gpart", [1, H], F32, isOutput=True)

    LR = mybir.ActivationFunctionType.Prelu
    CP = mybir.ActivationFunctionType.Copy

    with tile.TileContext(nc, num_cores=n_cores) as tc:
        with (
            tc.tile_pool(name="const", bufs=1) as cp,
            tc.tile_pool(name="persist", bufs=1) as pp,
            tc.tile_pool(name="dram", bufs=1, space="DRAM") as dp,
        ):
            # constants into SBUF
            cs = {}
            for nm in ["nb1", "eb1", "e2b1", "c1_b1", "c2_b1",
                       "iotaFb", "idn", "dstB"]:
                t = cp.tile(list(inp[nm].shape), F32, name=f"cs_{nm}", tag=f"cs_{nm}")
                nc.sync.dma_start(out=t[:], in_=inp[nm][:])
                cs[nm] = t
            for nm in ["nw1", "nw2a", "ew1", "ew2a", "e2w1", "e2w2a",
                       "c1_w1e", "c2_w1e",
                       "c1_w1n", "c1_w2a", "c2_w1n", "c2_w2a", "idnb"]:
                t = cp.tile(list(inp[nm].shape), BF16, name=f"cs_{nm}", tag=f"cs_{nm}")
                nc.sync.dma_start(out=t[:], in_=inp[nm][:])
                cs[nm] = t

            # persistent activations
            P1_sb = pp.tile([128, nw * H1], BF16, name="P1", tag="P1")
            P2_sb = pp.tile([128, nw * H1], BF16, name="P2", tag="P2")
            n1T_sb = pp.tile([128, np_pad], BF16, name="n1T", tag="n1T")

            # collective bounce buffers
            n1t_in = dp.tile([128, np_pad], F32, name="n1t_in", tag="n1t_in")
            n1t_ar = dp.tile([128, np_pad], F32, name="n1t_ar", tag="n1t_ar")
            n2_in = dp.tile([np_pad, 128], F32, name="n2_in", tag="n2_in")
            n2_rs = dp.tile([rs_rows, 128], F32, name="n2_rs", tag="n2_rs")

            # ---------------- phase: node MLP + P1
            with (
                tc.tile_pool(name="nmlp", bufs=3) as sp,
                tc.tile_pool(name="nmlp_ps", bufs=2, space="PSUM") as ps,
            ):
                for g0 in range(0, nw, 4):          # 4 windows = 512 nodes/tile
                    wn = min(4, nw - g0)
                    L = wn * WIN
                    nf = sp.tile([IN_NODE, L], BF16, tag="nf")
                    nc.sync.dma_start(out=nf[:], in_=inp["nfT"][:, g0 * WIN:g0 * WIN + L])
                    ph = ps.tile([H1, TS], F32, tag="ph")
                    nc.tensor.matmul(out=ph[:, :L], lhsT=cs["nw1"][:],
                                     rhs=nf[:], start=True, stop=True)
                    hh = sp.tile([H1A, L], BF16, tag="hh")
                    nc.scalar.activation(hh[:H1, :], ph[:, :L], LR, bias=cs["nb1"][:],
                                         alpha=SLOPE)
                    nc.gpsimd.memset(hh[H1:H1A, :], 1.0)
                    pn = ps.tile([H, TS], F32, tag="pn")
                    nc.tensor.matmul(out=pn[:, :L], lhsT=cs["nw2a"][:],
                                     rhs=hh[:], start=True, stop=True)
                    nT = sp.tile([H, L], BF16, tag="nT")
                    nc.scalar.activation(nT[:], pn[:, :L], LR, alpha=SLOPE)
                    for k in range(wn):
                        w = g0 + k
                        pP = ps.tile([128, H1], F32, tag="pP")
                        nc.tensor.matmul(out=pP[:], lhsT=nT[:, k * WIN:(k + 1) * WIN],
                                         rhs=cs["c1_w1n"][:], start=True, stop=True)
                        nc.scalar.activation(P1_sb[:, w * H1:(w + 1) * H1], pP[:], CP)

            # ---------------- conv phases
            def conv(ci, P_sb, bounce, transposed_scatter, emit_e, with_e2):
                w1e = cs[f"c{ci}_w1e"]
                b1 = cs[f"c{ci}_b1"]
                w2a = cs[f"c{ci}_w2a"]
                win_psum = {}
                with (
                    tc.tile_pool(name=f"cv{ci}", bufs=3) as sp,
                    tc.tile_pool(name=f"cv{ci}_psA", bufs=2, space="PSUM") as psA,
                    tc.tile_pool(name=f"cv{ci}_psB", bufs=1, space="PSUM") as psB,
                    tc.tile_pool(name=f"cv{ci}_psM", bufs=2, space="PSUM") as psM,
                    tc.tile_pool(name=f"cv{ci}_psW", bufs=2, space="PSUM") as psW,
                    tc.tile_pool(name=f"cv{ci}_psT", bufs=1, space="PSUM") as psT,
                ):
                    for t in range(nt):
                        ef = sp.tile([IN_EDGE, TS], BF16, tag="ef")
                        nc.sync.dma_start(out=ef[:], in_=inp["efT"][:, t * TS:(t + 1) * TS])
                        # e MLP
                        phe = psA.tile([H1, TS], F32, tag="psA")
                        nc.tensor.matmul(out=phe[:], lhsT=cs["ew1"][:],
                                         rhs=ef[:], start=True, stop=True)
                        he = sp.tile([H1A, TS], BF16, tag="he")
                        nc.scalar.activation(he[:H1, :], phe[:], LR, bias=cs["eb1"][:],
                                             alpha=SLOPE)
                        nc.gpsimd.memset(he[H1:H1A, :], 1.0)
                        pe = psB.tile([H, TS], F32, tag="psB")
                        nc.tensor.matmul(out=pe[:], lhsT=cs["ew2a"][:],
                                         rhs=he[:], start=True, stop=True)
                        eT = sp.tile([H, TS], BF16, tag="eT")
                        nc.scalar.activation(eT[:], pe[:], LR, alpha=SLOPE)
                        if emit_e:
                            nc.sync.dma_start(out=eT_out[:, t * TS:(t + 1) * TS],
                                              in_=eT[:])
                        if with_e2:
                            ph2 = psA.tile([H1, TS], F32, tag="psA")
                            nc.tensor.matmul(out=ph2[:], lhsT=cs["e2w1"][:],
                                             rhs=eT[:], start=True, stop=True)
                            h2 = sp.tile([H1A, TS], BF16, tag="he")
                            nc.scalar.activation(h2[:H1, :], ph2[:], LR,
                                                 bias=cs["e2b1"][:], alpha=SLOPE)
                            nc.gpsimd.memset(h2[H1:H1A, :], 1.0)
                            pe2 = psB.tile([H, TS], F32, tag="psB")
                            nc.tensor.matmul(out=pe2[:], lhsT=cs["e2w2a"][:],
                                             rhs=h2[:], start=True, stop=True)
                            eU = sp.tile([H, TS], BF16, tag="e2T")
                            nc.scalar.activation(eU[:], pe2[:], LR, alpha=SLOPE)
                        else:
                            eU = eT
                        # selection matrices for this tile's chunks:
                        # sels[e, m] = (dstoff[e] == m) built on DVE;
                        # selg = sels^T via PE transpose (for the gather).
                        sel_s, sel_g = [], []
                        for j in range(per_tile):
                            ch = t * per_tile + j
                            sels = sp.tile([128, 128], BF16, tag="sels",
                                           bufs=2 * per_tile)
                            nc.vector.tensor_tensor(
                                out=sels[:],
                                in0=cs["dstB"][:, ch:ch + 1].to_broadcast([128, 128]),
                                in1=cs["iotaFb"][:],
                                op=mybir.AluOpType.is_equal)
                            pT = psT.tile([128, 128], BF16, tag="psT")
                            nc.tensor.transpose(out=pT[:], in_=sels[:],
                                                identity=cs["idnb"][:])
                            selg = sp.tile([128, 128], BF16, tag="selg",
                                           bufs=2 * per_tile)
                            nc.vector.tensor_copy(out=selg[:], in_=pT[:])
                            sel_s.append(sels)
                            sel_g.append(selg)
                        # msg L1: e-part writes whole tile (incl ones row), then
                        # the per-chunk gather of P[dst] accumulates on top.
                        ph1 = psA.tile([H1, TS], F32, tag="psA")
                        nc.tensor.matmul(out=ph1[:], lhsT=w1e[:],
                                         rhs=eU[:], start=True, stop=False)
                        for j in range(per_tile):
                            ch = t * per_tile + j
                            w = int(win_of_chunk[ch])
                            nc.tensor.matmul(
                                out=ph1[:, j * 128:(j + 1) * 128],
                                lhsT=P_sb[:, w * H1:(w + 1) * H1],
                                rhs=sel_g[j][:], start=False,
                                stop=(j == per_tile - 1))
                        h1 = sp.tile([H1A, TS], BF16, tag="h1")
                        nc.scalar.activation(h1[:H1, :], ph1[:], LR, bias=b1[:],
                                             alpha=SLOPE)
                        nc.gpsimd.memset(h1[H1:H1A, :], 1.0)
                        # per chunk: msg L2 + scatter
                        for j in range(per_tile):
                            ch = t * per_tile + j
                            w = int(win_of_chunk[ch])
                            pmsg = psM.tile([128, 128], F32, tag="psM",
                                            name=f"pmsg{ci}_{t}_{j}")
                            nc.tensor.matmul(out=pmsg[:],
                                             lhsT=h1[:, j * 128:(j + 1) * 128],
                                             rhs=w2a[:], start=True, stop=True)
                            msg = sp.tile([128, 128], BF16, tag="msg")
                            nc.scalar.activation(msg[:], pmsg[:],
                                                 LR, alpha=SLOPE)
                            sels = sel_s[j]
                            first = ch == int(chunk0[w])
                            last = ch == int(chunk0[w]) + int(C[w]) - 1
                            if first:
                                win_psum[w] = psW.tile([128, 128], F32, tag="psW",
                                                       name=f"win{ci}_{w}")
                            pw = win_psum[w]
                            if transposed_scatter:
                                nc.tensor.matmul(out=pw[:], lhsT=msg[:], rhs=sels[:],
                                                 start=first, stop=last)
                            else:
                                nc.tensor.matmul(out=pw[:], lhsT=sels[:], rhs=msg[:],
                                                 start=first, stop=last)
                            if last:
                                st = sp.tile([128, 128], F32, tag="st")
                                nc.vector.tensor_copy(out=st[:], in_=pw[:])
                                if transposed_scatter:
                                    nc.sync.dma_start(
                                        out=bounce[:, w * WIN:(w + 1) * WIN],
                                        in_=st[:])
                                else:
                                    nc.sync.dma_start(
                                        out=bounce[w * WIN:(w + 1) * WIN, :],
                                        in_=st[:])
                                del win_psum[w]

            conv(1, P1_sb, n1t_in, transposed_scatter=True, emit_e=True,
                 with_e2=False)

            # AllReduce n1 (transposed layout), then P2 precompute
            nc.gpsimd.collective_compute(
                "AllReduce", mybir.AluOpType.add,
                replica_groups=[core_ids],
                ins=[n1t_in[:].opt()], outs=[n1t_ar[:].opt()])
            nc.gpsimd.dma_start(out=n1T_sb[:], in_=n1t_ar[:])  # f32 -> bf16 cast
            with tc.tile_pool(name="p2_ps", bufs=2, space="PSUM") as ps:
                for w in range(nw):
                    pP = ps.tile([128, H1], F32, tag="pP2")
                    nc.tensor.matmul(out=pP[:],
                                     lhsT=n1T_sb[:, w * WIN:(w + 1) * WIN],
                                     rhs=cs["c2_w1n"][:], start=True, stop=True)
                    nc.scalar.activation(P2_sb[:, w * H1:(w + 1) * H1], pP[:], CP)

            conv(2, P2_sb, n2_in, transposed_scatter=False, emit_e=False,
                 with_e2=True)

            # ReduceScatter n2 (natural layout) -> graph head shard
            nc.gpsimd.collective_compute(
                "ReduceScatter", mybir.AluOpType.add,
                replica_groups=[core_ids],
                ins=[n2_in[:].opt()], outs=[n2_rs[:].opt()])

            # ---------------- graph head
            with (
                tc.tile_pool(name="gh", bufs=4) as sp,
                tc.tile_pool(name="gh_ps", bufs=2, space="PSUM") as ps,
                tc.tile_pool(name="gh_acc", bufs=1, space="PSUM") as psacc,
            ):
                x_sb = pp.tile([128, rs_rows], BF16, name="xT", tag="xT")
                for b0 in range(0, rs_rows, 128):
                    L = min(128, rs_rows - b0)
                    ld = sp.tile([128, 128], F32, tag="ld")
                    nc.sync.dma_start(out=ld[:L, :], in_=n2_rs[b0:b0 + L, :])
                    pt = ps.tile([128, 128], F32, tag="pt")
                    nc.tensor.transpose(out=pt[:, :L], in_=ld[:L, :],
                                        identity=cs["idn"][:L, :L])
                    nc.scalar.activation(x_sb[:, b0:b0 + L], pt[:, :L], CP)
                nmm = rs_rows  # one matmul per 128-element x chunk (= one node)
                pg = psacc.tile([1, H], F32, tag="pg")
                for g in range(0, nmm, 4):
                    gw_t = sp.tile([128, 4 * H], BF16, tag="gw")
                    nc.sync.dma_start(out=gw_t[:],
                                      in_=inp["gws"][:, g * H:(g + 4) * H])
                    for j in range(4):
                        i = g + j
                        nc.tensor.matmul(out=pg[:], lhsT=x_sb[:, i:i + 1],
                                         rhs=gw_t[:, j * H:(j + 1) * H],
                                         start=(i == 0), stop=(i == nmm - 1))
                go = pp.tile([1, H], F32, name="go", tag="go")
                nc.scalar.activation(go[:], pg[:], CP)
                nc.sync.dma_start(out=gpart[:], in_=go[:])

    nc.finalize()
    return nc


# ---------------------------------------------------------------- entry point

_CACHE = {}


def _run(inputs, trace=False):
    inputs = {k: np.asarray(v) for k, v in inputs.items()}
    shared, per_core, meta = host_arrays(inputs, N, E, NCORES)
    key = hash(inputs["edge_index"].tobytes())
    if key not in _CACHE:
        _CACHE[key] = build_program(meta, NCORES)
    nc = _CACHE[key]
    in_maps = []
    for c in range(NCORES):
        m = dict(shared)
        m.update({k: v for k, v in per_core[c].items() if k != "slot_global"})
        in_maps.append(m)
    res = run_bass_kernel_spmd(nc, in_maps, list(range(NCORES)), trace=trace)

    # host-side unshard
    e_full = np.empty((E, H), np.float32)
    gsum = np.zeros(H, np.float32)
    for c in range(NCORES):
        out = res.results[c]
        sg = per_core[c]["slot_global"]
        mvalid = sg >= 0
        e_full[sg[mvalid]] = out["eT_out"].astype(np.float32).T[mvalid]
        gsum += out["gpart"][0]
    graph = gsum + inputs["graph_b"]
    graph = np.where(graph >= 0, graph, SLOPE * graph).astype(np.float32)
    return (graph, e_full), res


def kernel(**inputs):
    out, _ = _run(inputs, trace=False)
    return out


def kernel_profiled(**inputs):
    out, res = _run(inputs, trace=True)
    return out, res.exec_time_ns


# revision 20
# speedup vs baseline: 1.1768x; 1.1768x over previous
"""Trainium2 Bass kernel for nn_Encoder (GNN message passing), 8-core SPMD.

Strategy (edge-parallel):
  - Shard E edges across 8 cores; each core sorts its edges by dst node and
    processes them in 128-node "windows" so scatter-add becomes PSUM-matmul
    accumulation against on-device one-hot selection matrices.
  - Gather n[dst] is fused into the message MLP layer 1 by precomputing
    P = n @ w1_node per node; the gather matmul accumulates P[dst] directly
    into the L1 PSUM tile.
  - conv1 node sums accumulate transposed ([H, node]) and are AllReduced;
    conv2 sums accumulate natural ([node, H]) and are ReduceScattered to
    feed the input-dim-sharded graph head (graph_w in bf16, streamed).
  - e / e2 / node MLPs run in float32r; conv machinery runs in bf16 with
    fp32 PSUM accumulation.
"""

import math
import numpy as np
import ml_dtypes

import concourse.bacc as bacc
import concourse.bass as bass
import concourse.mybir as mybir
import concourse.tile as tile
from concourse.bass_utils import run_bass_kernel_spmd

F32 = mybir.dt.float32
F32R = mybir.dt.float32r
BF16 = mybir.dt.bfloat16

# problem constants (full size)
N = 10000
E = 320000
H = 128
NCORES = 8
IN_NODE = 4
IN_EDGE = 3
SLOPE = 0.1
WIN = 128          # nodes per scatter window
TS = 512           # edge slots per streaming tile
H1 = 64            # hidden dim of the 2-layer MLPs
H1A = H1 + 1       # augmented with ones-row for bias folding


# ---------------------------------------------------------------- host prep

def build_schedule(dst, n_nodes, n_edges, ncores):
    """Chunk schedule identical across cores, with edges dealt round-robin
    per destination window so per-(core,window) counts are balanced.

    Returns (C, win_of_chunk, chunk0_of_win, EP, slots) where slots[c] is an
    int64 array [EP] holding the GLOBAL edge index for each slot or -1.
    """
    nw = math.ceil(n_nodes / WIN)
    order = np.argsort(dst, kind="stable")       # all edges by dst
    wins = dst[order] // WIN
    cnt_all = np.bincount(wins, minlength=nw)
    # per-core count for window w: split cnt_all[w] as evenly as possible
    percore = -(-cnt_all[None, :] // ncores)     # ceil
    C = np.maximum(1, np.ceil(percore.max(axis=0) / 128).astype(np.int64))
    per_tile = TS // 128
    pad = (-int(C.sum())) % per_tile
    C[nw - 1] += pad
    nch = int(C.sum())
    ep = nch * 128
    chunk0 = np.zeros(nw, np.int64)
    chunk0[1:] = np.cumsum(C)[:-1]
    win_of_chunk = np.repeat(np.arange(nw), C)
    slots = [np.full(ep, -1, np.int64) for _ in range(ncores)]
    w0 = np.zeros(nw + 1, np.int64)
    w0[1:] = np.cumsum(cnt_all)
    for w in range(nw):
        ge = order[w0[w]:w0[w + 1]]              # global edges of window w
        base = 128 * int(chunk0[w])
        for c in range(ncores):
            mine = ge[c::ncores]
            slots[c][base:base + len(mine)] = mine
    return C, win_of_chunk, chunk0, ep, slots


def host_arrays(inputs, n_nodes, n_edges, ncores):
    """Build all per-core and shared device input arrays."""
    e_loc = n_edges // ncores
    nw = math.ceil(n_nodes / WIN)
    np_pad = nw * WIN
    dst = np.asarray(inputs["edge_index"][1])
    C, win_of_chunk, chunk0, ep, slots = build_schedule(dst, n_nodes, n_edges, ncores)
    nch = ep // 128
    ga = math.ceil(nch / 128)

    bf = lambda x: np.ascontiguousarray(x).astype(ml_dtypes.bfloat16)
    f32 = lambda x: np.ascontiguousarray(x, np.float32)

    shared = {}
    # node MLP (float32r path, stored f32)
    shared["nw1"] = bf(inputs["node_w1"])                 # [4,64]
    shared["nb1"] = f32(np.asarray(inputs["node_b1"])[:, None])  # [64,1]
    shared["nw2a"] = bf(np.concatenate(
        [inputs["node_w2"], inputs["node_b2"][None, :]], axis=0))  # [65,128]
    # edge MLP
    shared["ew1"] = bf(inputs["edge_w1"])                 # [3,64]
    shared["eb1"] = f32(np.asarray(inputs["edge_b1"])[:, None])
    shared["ew2a"] = bf(np.concatenate(
        [inputs["edge_w2"], inputs["edge_b2"][None, :]], axis=0))  # [65,128]
    # e2 MLP (last=True edge transform)
    shared["e2w1"] = bf(inputs["g2e_w1"])                 # [128,64]
    shared["e2b1"] = f32(np.asarray(inputs["g2e_b1"])[:, None])
    shared["e2w2a"] = bf(np.concatenate(
        [inputs["g2e_w2"], inputs["g2e_b2"][None, :]], axis=0))  # [65,128]
    # conv weights (bf16 machinery)
    for i, pre in ((1, "g1"), (2, "g2")):
        w1 = np.asarray(inputs[f"{pre}_w1"])  # [2H, 64]
        shared[f"c{i}_w1n"] = bf(w1[:H])                      # [128,64]
        shared[f"c{i}_w1e"] = bf(w1[H:])                     # [128,64] f32r
        shared[f"c{i}_b1"] = f32(np.asarray(inputs[f"{pre}_b1"])[:, None])  # [64,1]
        shared[f"c{i}_w2a"] = bf(np.concatenate(
            [inputs[f"{pre}_w2"], inputs[f"{pre}_b2"][None, :]], axis=0))  # [65,128]
    # consts
    shared["iotaFb"] = bf(np.tile(np.arange(128, dtype=np.float32), (128, 1)))
    shared["idn"] = f32(np.eye(128, dtype=np.float32))
    shared["idnb"] = bf(np.eye(128, dtype=np.float32))
    # node features, padded + transposed
    nf = np.zeros((np_pad, IN_NODE), np.float32)
    nf[:n_nodes] = inputs["node_feat"]
    shared["nfT"] = bf(nf.T)  # [4, NP]

    # per-core arrays
    per_core = []
    ef = np.asarray(inputs["edge_feat"])
    rs_rows = np_pad // ncores
    gw = np.zeros((np_pad * H, H), np.float32)
    gw[:n_nodes * H] = inputs["graph_w"]
    for c in range(ncores):
        slot = slots[c]
        valid = slot >= 0
        eft = np.zeros((IN_EDGE, ep), np.float32)
        eft[:, valid] = ef[slot[valid]].T
        eft = bf(eft)
        dstoff = np.full(ep, -1.0, np.float32)
        dstoff[valid] = (dst[slot[valid]] - WIN * win_of_chunk[
            np.arange(ep) // 128][valid]).astype(np.float32)
        dstB = bf(dstoff.reshape(nch, 128).T)          # [128, NCH] bf16
        # graph head weight shard, shuffled so rhs tiles DMA contiguously:
        # gws[k, 128*i + h] = gw_shard[128*i + k, h]
        gshard = gw[c * rs_rows * H:(c + 1) * rs_rows * H]   # [rs_rows*H, H]
        nchunk = gshard.shape[0] // 128
        gws = bf(gshard.reshape(nchunk, 128, H).transpose(1, 0, 2).reshape(128, -1))
        per_core.append({
            "efT": eft, "dstB": dstB, "gws": gws,
            "slot_global": slot,
        })
    meta = dict(C=C, win_of_chunk=win_of_chunk, chunk0=chunk0, ep=ep,
                nch=nch, ga=ga, nw=nw, np_pad=np_pad, e_loc=e_loc,
                rs_rows=rs_rows)
    return shared, per_core, meta


# ---------------------------------------------------------------- device prog

def build_program(meta, n_cores):
    nw, np_pad, ep, nch, ga = (meta["nw"], meta["np_pad"], meta["ep"],
                               meta["nch"], meta["ga"])
    C, win_of_chunk, chunk0 = meta["C"], meta["win_of_chunk"], meta["chunk0"]
    rs_rows = meta["rs_rows"]
    nt = ep // TS
    per_tile = TS // 128

    nc = bacc.Bacc(num_devices=n_cores)
    core_ids = list(range(n_cores))

    # ---- I/O declarations
    inp = {}
    def din(name, shape, dtype=F32):
        inp[name] = nc.declare_dram_parameter(name, list(shape), dtype, isOutput=False)
        return inp[name]

    for nm, sh in [
        ("nb1", (H1, 1)), ("eb1", (H1, 1)), ("e2b1", (H1, 1)),
        ("c1_b1", (H1, 1)), ("c2_b1", (H1, 1)),
        ("idn", (128, 128)),
    ]:
        din(nm, sh)
    din("iotaFb", (128, 128), BF16)
    din("dstB", (128, nch), BF16)
    for nm, sh in [("nw1", (IN_NODE, H1)), ("nw2a", (H1A, H)),
                   ("ew1", (IN_EDGE, H1)), ("ew2a", (H1A, H)),
                   ("e2w1", (H, H1)), ("e2w2a", (H1A, H)),
                   ("c1_w1e", (H, H1)), ("c2_w1e", (H, H1)),
                   ("c1_w1n", (H, H1)), ("c1_w2a", (H1A, H)),
                   ("c2_w1n", (H, H1)), ("c2_w2a", (H1A, H)),
                   ("idnb", (128, 128)), ("nfT", (IN_NODE, np_pad)),
                   ("efT", (IN_EDGE, ep)),
                   ("gws", (128, rs_rows * H))]:
        din(nm, sh, BF16)

    eT_out = nc.declare_dram_parameter("eT_out", [128, ep], BF16, isOutput=True)
    gpart = nc.declare_dram_parameter("gpart", [1, H], F32, isOutput=True)

    LR = mybir.ActivationFunctionType.Prelu
    CP = mybir.ActivationFunctionType.Copy

    with tile.TileContext(nc, num_cores=n_cores) as tc:
        with (
            tc.tile_pool(name="const", bufs=1) as cp,
            tc.tile_pool(name="persist", bufs=1) as pp,
            tc.tile_pool(name="dram", bufs=1, space="DRAM") as dp,
        ):
            # constants into SBUF
            cs = {}
            for nm in ["nb1", "eb1", "e2b1", "c1_b1", "c2_b1", "idn"]:
                t = cp.tile(list(inp[nm].shape), F32, name=f"cs_{nm}", tag=f"cs_{nm}")
                nc.sync.dma_start(out=t[:], in_=inp[nm][:])
                cs[nm] = t
            for nm in ["nw1", "nw2a", "ew1", "ew2a", "e2w1", "e2w2a",
                       "c1_w1e", "c2_w1e", "iotaFb", "dstB",
                       "c1_w1n", "c1_w2a", "c2_w1n", "c2_w2a", "idnb"]:
                t = cp.tile(list(inp[nm].shape), BF16, name=f"cs_{nm}", tag=f"cs_{nm}")
                nc.sync.dma_start(out=t[:], in_=inp[nm][:])
                cs[nm] = t

            # persistent activations
            P1_sb = pp.tile([128, nw * H1], BF16, name="P1", tag="P1")
            P2_sb = pp.tile([128, nw * H1], BF16, name="P2", tag="P2")
            n1T_sb = pp.tile([128, np_pad], BF16, name="n1T", tag="n1T")

            # collective bounce buffers
            n1t_in = dp.tile([128, np_pad], F32, name="n1t_in", tag="n1t_in")
            n1t_ar = dp.tile([128, np_pad], F32, name="n1t_ar", tag="n1t_ar")
            n2_in = dp.tile([np_pad, 128], F32, name="n2_in", tag="n2_in")
            n2_rs = dp.tile([rs_rows, 128], F32, name="n2_rs", tag="n2_rs")

            # ---------------- phase: node MLP + P1
            with (
                tc.tile_pool(name="nmlp", bufs=3) as sp,
                tc.tile_pool(name="nmlp_ps", bufs=2, space="PSUM") as ps,
            ):
                for g0 in range(0, nw, 4):          # 4 windows = 512 nodes/tile
                    wn = min(4, nw - g0)
                    L = wn * WIN
                    nf = sp.tile([IN_NODE, L], BF16, tag="nf")
                    nc.sync.dma_start(out=nf[:], in_=inp["nfT"][:, g0 * WIN:g0 * WIN + L])
                    ph = ps.tile([H1, TS], F32, tag="ph")
                    nc.tensor.matmul(out=ph[:, :L], lhsT=cs["nw1"][:],
                                     rhs=nf[:], start=True, stop=True)
                    hh = sp.tile([H1A, L], BF16, tag="hh")
                    nc.scalar.activation(hh[:H1, :], ph[:, :L], LR, bias=cs["nb1"][:],
                                         alpha=SLOPE)
                    nc.gpsimd.memset(hh[H1:H1A, :], 1.0)
                    pn = ps.tile([H, TS], F32, tag="pn")
                    nc.tensor.matmul(out=pn[:, :L], lhsT=cs["nw2a"][:],
                                     rhs=hh[:], start=True, stop=True)
                    nT = sp.tile([H, L], BF16, tag="nT")
                    nc.scalar.activation(nT[:], pn[:, :L], LR, alpha=SLOPE)
                    for k in range(wn):
                        w = g0 + k
                        pP = ps.tile([128, H1], F32, tag="pP")
                        nc.tensor.matmul(out=pP[:], lhsT=nT[:, k * WIN:(k + 1) * WIN],
                                         rhs=cs["c1_w1n"][:], start=True, stop=True)
                        nc.scalar.activation(P1_sb[:, w * H1:(w + 1) * H1], pP[:], CP)

            # ---------------- conv phases
            def conv(ci, P_sb, bounce, transposed_scatter, emit_e, with_e2):
                w1e = cs[f"c{ci}_w1e"]
                b1 = cs[f"c{ci}_b1"]
                w2a = cs[f"c{ci}_w2a"]
                win_psum = {}
                with (
                    tc.tile_pool(name=f"cv{ci}", bufs=3) as sp,
                    tc.tile_pool(name=f"cv{ci}_psA", bufs=2, space="PSUM") as psA,
                    tc.tile_pool(name=f"cv{ci}_psB", bufs=1, space="PSUM") as psB,
                    tc.tile_pool(name=f"cv{ci}_psM", bufs=2, space="PSUM") as psM,
                    tc.tile_pool(name=f"cv{ci}_psW", bufs=2, space="PSUM") as psW,
                    tc.tile_pool(name=f"cv{ci}_psT", bufs=1, space="PSUM") as psT,
                ):
                    for t in range(nt):
                        ef = sp.tile([IN_EDGE, TS], BF16, tag="ef")
                        nc.sync.dma_start(out=ef[:], in_=inp["efT"][:, t * TS:(t + 1) * TS])
                        # e MLP
                        phe = psA.tile([H1, TS], F32, tag="psA")
                        nc.tensor.matmul(out=phe[:], lhsT=cs["ew1"][:],
                                         rhs=ef[:], start=True, stop=True)
                        he = sp.tile([H1A, TS], BF16, tag="he")
                        nc.scalar.activation(he[:H1, :], phe[:], LR, bias=cs["eb1"][:],
                                             alpha=SLOPE)
                        nc.gpsimd.memset(he[H1:H1A, :], 1.0)
                        pe = psB.tile([H, TS], F32, tag="psB")
                        nc.tensor.matmul(out=pe[:], lhsT=cs["ew2a"][:],
                                         rhs=he[:], start=True, stop=True)
                        eT = sp.tile([H, TS], BF16, tag="eT")
                        nc.scalar.activation(eT[:], pe[:], LR, alpha=SLOPE)
                        if emit_e:
                            nc.sync.dma_start(out=eT_out[:, t * TS:(t + 1) * TS],
                                              in_=eT[:])
                        if with_e2:
                            ph2 = psA.tile([H1, TS], F32, tag="psA")
                            nc.tensor.matmul(out=ph2[:], lhsT=cs["e2w1"][:],
                                             rhs=eT[:], start=True, stop=True)
                            h2 = sp.tile([H1A, TS], BF16, tag="he")
                            nc.scalar.activation(h2[:H1, :], ph2[:], LR,
                                                 bias=cs["e2b1"][:], alpha=SLOPE)
                            nc.gpsimd.memset(h2[H1:H1A, :], 1.0)
                            pe2 = psB.tile([H, TS], F32, tag="psB")
                            nc.tensor.matmul(out=pe2[:], lhsT=cs["e2w2a"][:],
                                             rhs=h2[:], start=True, stop=True)
                            eU = sp.tile([H, TS], BF16, tag="e2T")
                            nc.scalar.activation(eU[:], pe2[:], LR, alpha=SLOPE)
                        else:
                            eU = eT
                        # selection matrices for this tile's chunks:
                        # sels[e, m] = (dstoff[e] == m) built on DVE;
                        # selg = sels^T via PE transpose (for the gather).
                        sel_s = []
                        selg_t = sp.tile([128, TS], BF16, tag="selg", bufs=2)
                        for j in range(per_tile):
                            ch = t * per_tile + j
                            sels = sp.tile([128, 128], BF16, tag="sels",
                                           bufs=2 * per_tile)
                            nc.vector.tensor_tensor(
                                out=sels[:],
                                in0=cs["dstB"][:, ch:ch + 1].to_broadcast([128, 128]),
                                in1=cs["iotaFb"][:],
                                op=mybir.AluOpType.is_equal)
                            pT = psT.tile([128, 128], BF16, tag="psT")
                            nc.tensor.transpose(out=pT[:], in_=sels[:],
                                                identity=cs["idnb"][:])
                            nc.vector.tensor_copy(
                                out=selg_t[:, j * 128:(j + 1) * 128], in_=pT[:])
                            sel_s.append(sels)
                        # msg L1: e-part writes whole tile (incl ones row), then
                        # the per-chunk gather of P[dst] accumulates on top.
                        ph1 = psA.tile([H1, TS], F32, tag="psA")
                        nc.tensor.matmul(out=ph1[:], lhsT=w1e[:],
                                         rhs=eU[:], start=True, stop=False)
                        j = 0
                        while j < per_tile:
                            w = int(win_of_chunk[t * per_tile + j])
                            je = j
                            while (je + 1 < per_tile and
                                   int(win_of_chunk[t * per_tile + je + 1]) == w):
                                je += 1
                            nc.tensor.matmul(
                                out=ph1[:, j * 128:(je + 1) * 128],
                                lhsT=P_sb[:, w * H1:(w + 1) * H1],
                                rhs=selg_t[:, j * 128:(je + 1) * 128],
                                start=False, stop=(je == per_tile - 1))
                            j = je + 1
                        h1 = sp.tile([H1A, TS], BF16, tag="h1")
                        nc.scalar.activation(h1[:H1, :], ph1[:], LR, bias=b1[:],
                                             alpha=SLOPE)
                        nc.gpsimd.memset(h1[H1:H1A, :], 1.0)
                        # per chunk: msg L2 + scatter
                        for j in range(per_tile):
                            ch = t * per_tile + j
                            w = int(win_of_chunk[ch])
                            pmsg = psM.tile([128, 128], F32, tag="psM",
                                            name=f"pmsg{ci}_{t}_{j}")
                            nc.tensor.matmul(out=pmsg[:],
                                             lhsT=h1[:, j * 128:(j + 1) * 128],
                                             rhs=w2a[:], start=True, stop=True)
                            msg = sp.tile([128, 128], BF16, tag="msg")
                            nc.scalar.activation(msg[:], pmsg[:],
                                                 LR, alpha=SLOPE)
                            sels = sel_s[j]
                            first = ch == int(chunk0[w])
                            last = ch == int(chunk0[w]) + int(C[w]) - 1
                            if first:
                                win_psum[w] = psW.tile([128, 128], F32, tag="psW",
                                                       name=f"win{ci}_{w}")
                            pw = win_psum[w]
                            if transposed_scatter:
                                nc.tensor.matmul(out=pw[:], lhsT=msg[:], rhs=sels[:],
                                                 start=first, stop=last)
                            else:
                                nc.tensor.matmul(out=pw[:], lhsT=sels[:], rhs=msg[:],
                                                 start=first, stop=last)
                            if last:
                                st = sp.tile([128, 128], F32, tag="st")
                                nc.vector.tensor_copy(out=st[:], in_=pw[:])
                                if transposed_scatter:
                                    nc.sync.dma_start(
                                        out=bounce[:, w * WIN:(w + 1) * WIN],
                                        in_=st[:])
                                else:
                                    nc.sync.dma_start(
                                        out=bounce[w * WIN:(w + 1) * WIN, :],
                                        in_=st[:])
                                del win_psum[w]

            conv(1, P1_sb, n1t_in, transposed_scatter=True, emit_e=True,
                 with_e2=False)

            # AllReduce n1 (transposed layout), then P2 precompute
            nc.gpsimd.collective_compute(
                "AllReduce", mybir.AluOpType.add,
                replica_groups=[core_ids],
                ins=[n1t_in[:].opt()], outs=[n1t_ar[:].opt()])
            nc.gpsimd.dma_start(out=n1T_sb[:], in_=n1t_ar[:])  # f32 -> bf16 cast
            with tc.tile_pool(name="p2_ps", bufs=2, space="PSUM") as ps:
                for w in range(nw):
                    pP = ps.tile([128, H1], F32, tag="pP2")
                    nc.tensor.matmul(out=pP[:],
                                     lhsT=n1T_sb[:, w * WIN:(w + 1) * WIN],
                                     rhs=cs["c2_w1n"][:], start=True, stop=True)
                    nc.scalar.activation(P2_sb[:, w * H1:(w + 1) * H1], pP[:], CP)

            conv(2, P2_sb, n2_in, transposed_scatter=False, emit_e=False,
                 with_e2=True)

            # ReduceScatter n2 (natural layout) -> graph head shard
            nc.gpsimd.collective_compute(
                "ReduceScatter", mybir.AluOpType.add,
                replica_groups=[core_ids],
                ins=[n2_in[:].opt()], outs=[n2_rs[:].opt()])

            # ---------------- graph head
            with (
                tc.tile_pool(name="gh", bufs=4) as sp,
                tc.tile_pool(name="gh_ps", bufs=2, space="PSUM") as ps,
                tc.tile_pool(name="gh_acc", bufs=1, space="PSUM") as psacc,
            ):
                x_sb = pp.tile([128, rs_rows], BF16, name="xT", tag="xT")
                for b0 in range(0, rs_rows, 128):
                    L = min(128, rs_rows - b0)
                    ld = sp.tile([128, 128], F32, tag="ld")
                    nc.sync.dma_start(out=ld[:L, :], in_=n2_rs[b0:b0 + L, :])
                    pt = ps.tile([128, 128], F32, tag="pt")
                    nc.tensor.transpose(out=pt[:, :L], in_=ld[:L, :],
                                        identity=cs["idn"][:L, :L])
                    nc.scalar.activation(x_sb[:, b0:b0 + L], pt[:, :L], CP)
                nmm = rs_rows  # one matmul per 128-element x chunk (= one node)
                GB = 16        # graph chunks per DMA batch
                pg = psacc.tile([1, H], F32, tag="pg")
                for g in range(0, nmm, GB):
                    gn = min(GB, nmm - g)
                    gw_t = sp.tile([128, GB * H], BF16, tag="gw")
                    nc.sync.dma_start(out=gw_t[:, :gn * H],
                                      in_=inp["gws"][:, g * H:(g + gn) * H])
                    for j in range(gn):
                        i = g + j
                        nc.tensor.matmul(out=pg[:], lhsT=x_sb[:, i:i + 1],
                                         rhs=gw_t[:, j * H:(j + 1) * H],
                                         start=(i == 0), stop=(i == nmm - 1))
                go = pp.tile([1, H], F32, name="go", tag="go")
                nc.scalar.activation(go[:], pg[:], CP)
                nc.sync.dma_start(out=gpart[:], in_=go[:])

    nc.finalize()
    return nc


# ---------------------------------------------------------------- entry point

_CACHE = {}


def _run(inputs, trace=False):
    inputs = {k: np.asarray(v) for k, v in inputs.items()}
    shared, per_core, meta = host_arrays(inputs, N, E, NCORES)
    key = hash(inputs["edge_index"].tobytes())
    if key not in _CACHE:
        _CACHE[key] = build_program(meta, NCORES)
    nc = _CACHE[key]
    in_maps = []
    for c in range(NCORES):
        m = dict(shared)
        m.update({k: v for k, v in per_core[c].items() if k != "slot_global"})
        in_maps.append(m)
    res = run_bass_kernel_spmd(nc, in_maps, list(range(NCORES)), trace=trace)

    # host-side unshard
    e_full = np.empty((E, H), np.float32)
    gsum = np.zeros(H, np.float32)
    for c in range(NCORES):
        out = res.results[c]
        sg = per_core[c]["slot_global"]
        mvalid = sg >= 0
        e_full[sg[mvalid]] = out["eT_out"].astype(np.float32).T[mvalid]
        gsum += out["gpart"][0]
    graph = gsum + inputs["graph_b"]
    graph = np.where(graph >= 0, graph, SLOPE * graph).astype(np.float32)
    return (graph, e_full), res


def kernel(**inputs):
    out, _ = _run(inputs, trace=False)
    return out


def kernel_profiled(**inputs):
    out, res = _run(inputs, trace=True)
    return out, res.exec_time_ns


# revision 22
# speedup vs baseline: 1.1793x; 1.0021x over previous
"""Trainium2 Bass kernel for nn_Encoder (GNN message passing), 8-core SPMD.

Strategy (edge-parallel):
  - Shard E edges across 8 cores; each core sorts its edges by dst node and
    processes them in 128-node "windows" so scatter-add becomes PSUM-matmul
    accumulation against on-device one-hot selection matrices.
  - Gather n[dst] is fused into the message MLP layer 1 by precomputing
    P = n @ w1_node per node; the gather matmul accumulates P[dst] directly
    into the L1 PSUM tile.
  - conv1 node sums accumulate transposed ([H, node]) and are AllReduced;
    conv2 sums accumulate natural ([node, H]) and are ReduceScattered to
    feed the input-dim-sharded graph head (graph_w in bf16, streamed).
  - e / e2 / node MLPs run in float32r; conv machinery runs in bf16 with
    fp32 PSUM accumulation.
"""

import math
import numpy as np
import ml_dtypes

import concourse.bacc as bacc
import concourse.bass as bass
import concourse.mybir as mybir
import concourse.tile as tile
from concourse.bass_utils import run_bass_kernel_spmd

F32 = mybir.dt.float32
F32R = mybir.dt.float32r
BF16 = mybir.dt.bfloat16

# problem constants (full size)
N = 10000
E = 320000
H = 128
NCORES = 8
IN_NODE = 4
IN_EDGE = 3
SLOPE = 0.1
WIN = 128          # nodes per scatter window
TS = 512           # edge slots per streaming tile
H1 = 64            # hidden dim of the 2-layer MLPs
H1A = H1 + 1       # augmented with ones-row for bias folding


# ---------------------------------------------------------------- host prep

def build_schedule(dst, n_nodes, n_edges, ncores):
    """Chunk schedule identical across cores, with edges dealt round-robin
    per destination window so per-(core,window) counts are balanced.

    Returns (C, win_of_chunk, chunk0_of_win, EP, slots) where slots[c] is an
    int64 array [EP] holding the GLOBAL edge index for each slot or -1.
    """
    nw = math.ceil(n_nodes / WIN)
    order = np.argsort(dst, kind="stable")       # all edges by dst
    wins = dst[order] // WIN
    cnt_all = np.bincount(wins, minlength=nw)
    # per-core count for window w: split cnt_all[w] as evenly as possible
    percore = -(-cnt_all[None, :] // ncores)     # ceil
    C = np.maximum(1, np.ceil(percore.max(axis=0) / 128).astype(np.int64))
    per_tile = TS // 128
    pad = (-int(C.sum())) % per_tile
    C[nw - 1] += pad
    nch = int(C.sum())
    ep = nch * 128
    chunk0 = np.zeros(nw, np.int64)
    chunk0[1:] = np.cumsum(C)[:-1]
    win_of_chunk = np.repeat(np.arange(nw), C)
    slots = [np.full(ep, -1, np.int64) for _ in range(ncores)]
    w0 = np.zeros(nw + 1, np.int64)
    w0[1:] = np.cumsum(cnt_all)
    for w in range(nw):
        ge = order[w0[w]:w0[w + 1]]              # global edges of window w
        base = 128 * int(chunk0[w])
        for c in range(ncores):
            mine = ge[c::ncores]
            slots[c][base:base + len(mine)] = mine
    return C, win_of_chunk, chunk0, ep, slots


def host_arrays(inputs, n_nodes, n_edges, ncores):
    """Build all per-core and shared device input arrays."""
    e_loc = n_edges // ncores
    nw = math.ceil(n_nodes / WIN)
    np_pad = nw * WIN
    dst = np.asarray(inputs["edge_index"][1])
    C, win_of_chunk, chunk0, ep, slots = build_schedule(dst, n_nodes, n_edges, ncores)
    nch = ep // 128
    ga = math.ceil(nch / 128)

    bf = lambda x: np.ascontiguousarray(x).astype(ml_dtypes.bfloat16)
    f32 = lambda x: np.ascontiguousarray(x, np.float32)

    shared = {}
    # node MLP (float32r path, stored f32)
    shared["nw1"] = bf(inputs["node_w1"])                 # [4,64]
    shared["nb1"] = f32(np.asarray(inputs["node_b1"])[:, None])  # [64,1]
    shared["nw2a"] = bf(np.concatenate(
        [inputs["node_w2"], inputs["node_b2"][None, :]], axis=0))  # [65,128]
    # edge MLP
    shared["ew1"] = bf(inputs["edge_w1"])                 # [3,64]
    shared["eb1"] = f32(np.asarray(inputs["edge_b1"])[:, None])
    shared["ew2a"] = bf(np.concatenate(
        [inputs["edge_w2"], inputs["edge_b2"][None, :]], axis=0))  # [65,128]
    # e2 MLP (last=True edge transform)
    shared["e2w1"] = bf(inputs["g2e_w1"])                 # [128,64]
    shared["e2b1"] = f32(np.asarray(inputs["g2e_b1"])[:, None])
    shared["e2w2a"] = bf(np.concatenate(
        [inputs["g2e_w2"], inputs["g2e_b2"][None, :]], axis=0))  # [65,128]
    # conv weights (bf16 machinery)
    for i, pre in ((1, "g1"), (2, "g2")):
        w1 = np.asarray(inputs[f"{pre}_w1"])  # [2H, 64]
        shared[f"c{i}_w1n"] = bf(w1[:H])                      # [128,64]
        shared[f"c{i}_w1e"] = bf(w1[H:])                     # [128,64] f32r
        shared[f"c{i}_b1"] = f32(np.asarray(inputs[f"{pre}_b1"])[:, None])  # [64,1]
        shared[f"c{i}_w2a"] = bf(np.concatenate(
            [inputs[f"{pre}_w2"], inputs[f"{pre}_b2"][None, :]], axis=0))  # [65,128]
    # consts
    shared["iotaFb"] = bf(np.tile(np.arange(128, dtype=np.float32), (128, 1)))
    shared["idn"] = f32(np.eye(128, dtype=np.float32))
    shared["idnb"] = bf(np.eye(128, dtype=np.float32))
    # node features, padded + transposed
    nf = np.zeros((np_pad, IN_NODE), np.float32)
    nf[:n_nodes] = inputs["node_feat"]
    shared["nfT"] = bf(nf.T)  # [4, NP]

    # per-core arrays
    per_core = []
    ef = np.asarray(inputs["edge_feat"])
    rs_rows = np_pad // ncores
    gw = np.zeros((np_pad * H, H), np.float32)
    gw[:n_nodes * H] = inputs["graph_w"]
    for c in range(ncores):
        slot = slots[c]
        valid = slot >= 0
        eft = np.zeros((IN_EDGE, ep), np.float32)
        eft[:, valid] = ef[slot[valid]].T
        eft = bf(eft)
        dstoff = np.full(ep, -1.0, np.float32)
        dstoff[valid] = (dst[slot[valid]] - WIN * win_of_chunk[
            np.arange(ep) // 128][valid]).astype(np.float32)
        dstB = bf(dstoff.reshape(nch, 128).T)          # [128, NCH] bf16
        # graph head weight shard, shuffled so rhs tiles DMA contiguously:
        # gws[k, 128*i + h] = gw_shard[128*i + k, h]
        gshard = gw[c * rs_rows * H:(c + 1) * rs_rows * H]   # [rs_rows*H, H]
        nchunk = gshard.shape[0] // 128
        gws = bf(gshard.reshape(nchunk, 128, H).transpose(1, 0, 2).reshape(128, -1))
        per_core.append({
            "efT": eft, "dstB": dstB, "gws": gws,
            "slot_global": slot,
        })
    meta = dict(C=C, win_of_chunk=win_of_chunk, chunk0=chunk0, ep=ep,
                nch=nch, ga=ga, nw=nw, np_pad=np_pad, e_loc=e_loc,
                rs_rows=rs_rows)
    return shared, per_core, meta


# ---------------------------------------------------------------- device prog

def build_program(meta, n_cores):
    nw, np_pad, ep, nch, ga = (meta["nw"], meta["np_pad"], meta["ep"],
                               meta["nch"], meta["ga"])
    C, win_of_chunk, chunk0 = meta["C"], meta["win_of_chunk"], meta["chunk0"]
    rs_rows = meta["rs_rows"]
    nt = ep // TS
    per_tile = TS // 128

    nc = bacc.Bacc(num_devices=n_cores)
    core_ids = list(range(n_cores))

    # ---- I/O declarations
    inp = {}
    def din(name, shape, dtype=F32):
        inp[name] = nc.declare_dram_parameter(name, list(shape), dtype, isOutput=False)
        return inp[name]

    for nm, sh in [
        ("nb1", (H1, 1)), ("eb1", (H1, 1)), ("e2b1", (H1, 1)),
        ("c1_b1", (H1, 1)), ("c2_b1", (H1, 1)),
        ("idn", (128, 128)),
    ]:
        din(nm, sh)
    din("iotaFb", (128, 128), BF16)
    din("dstB", (128, nch), BF16)
    for nm, sh in [("nw1", (IN_NODE, H1)), ("nw2a", (H1A, H)),
                   ("ew1", (IN_EDGE, H1)), ("ew2a", (H1A, H)),
                   ("e2w1", (H, H1)), ("e2w2a", (H1A, H)),
                   ("c1_w1e", (H, H1)), ("c2_w1e", (H, H1)),
                   ("c1_w1n", (H, H1)), ("c1_w2a", (H1A, H)),
                   ("c2_w1n", (H, H1)), ("c2_w2a", (H1A, H)),
                   ("idnb", (128, 128)), ("nfT", (IN_NODE, np_pad)),
                   ("efT", (IN_EDGE, ep)),
                   ("gws", (128, rs_rows * H))]:
        din(nm, sh, BF16)

    eT_out = nc.declare_dram_parameter("eT_out", [128, ep], BF16, isOutput=True)
    gpart = nc.declare_dram_parameter("gpart", [1, H], F32, isOutput=True)

    LR = mybir.ActivationFunctionType.Prelu
    CP = mybir.ActivationFunctionType.Copy

    with tile.TileContext(nc, num_cores=n_cores) as tc:
        with (
            tc.tile_pool(name="const", bufs=1) as cp,
            tc.tile_pool(name="persist", bufs=1) as pp,
            tc.tile_pool(name="dram", bufs=1, space="DRAM") as dp,
        ):
            # constants into SBUF
            cs = {}
            for nm in ["nb1", "eb1", "e2b1", "c1_b1", "c2_b1", "idn"]:
                t = cp.tile(list(inp[nm].shape), F32, name=f"cs_{nm}", tag=f"cs_{nm}")
                nc.sync.dma_start(out=t[:], in_=inp[nm][:])
                cs[nm] = t
            for nm in ["nw1", "nw2a", "ew1", "ew2a", "e2w1", "e2w2a",
                       "c1_w1e", "c2_w1e", "iotaFb", "dstB",
                       "c1_w1n", "c1_w2a", "c2_w1n", "c2_w2a", "idnb"]:
                t = cp.tile(list(inp[nm].shape), BF16, name=f"cs_{nm}", tag=f"cs_{nm}")
                nc.sync.dma_start(out=t[:], in_=inp[nm][:])
                cs[nm] = t

            # persistent activations
            P1_sb = pp.tile([128, nw * H1], BF16, name="P1", tag="P1")
            P2_sb = pp.tile([128, nw * H1], BF16, name="P2", tag="P2")
            n1T_sb = pp.tile([128, np_pad], BF16, name="n1T", tag="n1T")

            # collective bounce buffers
            n1t_in = dp.tile([128, np_pad], F32, name="n1t_in", tag="n1t_in")
            n1t_ar = dp.tile([128, np_pad], F32, name="n1t_ar", tag="n1t_ar")
            n2_in = dp.tile([np_pad, 128], F32, name="n2_in", tag="n2_in")
            n2_rs = dp.tile([rs_rows, 128], F32, name="n2_rs", tag="n2_rs")

            # ---------------- phase: node MLP + P1
            with (
                tc.tile_pool(name="nmlp", bufs=3) as sp,
                tc.tile_pool(name="nmlp_ps", bufs=2, space="PSUM") as ps,
            ):
                for g0 in range(0, nw, 4):          # 4 windows = 512 nodes/tile
                    wn = min(4, nw - g0)
                    L = wn * WIN
                    nf = sp.tile([IN_NODE, L], BF16, tag="nf")
                    nc.sync.dma_start(out=nf[:], in_=inp["nfT"][:, g0 * WIN:g0 * WIN + L])
                    ph = ps.tile([H1, TS], F32, tag="ph")
                    nc.tensor.matmul(out=ph[:, :L], lhsT=cs["nw1"][:],
                                     rhs=nf[:], start=True, stop=True)
                    hh = sp.tile([H1A, L], BF16, tag="hh")
                    nc.scalar.activation(hh[:H1, :], ph[:, :L], LR, bias=cs["nb1"][:],
                                         alpha=SLOPE)
                    nc.gpsimd.memset(hh[H1:H1A, :], 1.0)
                    pn = ps.tile([H, TS], F32, tag="pn")
                    nc.tensor.matmul(out=pn[:, :L], lhsT=cs["nw2a"][:],
                                     rhs=hh[:], start=True, stop=True)
                    nT = sp.tile([H, L], BF16, tag="nT")
                    nc.scalar.activation(nT[:], pn[:, :L], LR, alpha=SLOPE)
                    for k in range(wn):
                        w = g0 + k
                        pP = ps.tile([128, H1], F32, tag="pP")
                        nc.tensor.matmul(out=pP[:], lhsT=nT[:, k * WIN:(k + 1) * WIN],
                                         rhs=cs["c1_w1n"][:], start=True, stop=True)
                        nc.scalar.activation(P1_sb[:, w * H1:(w + 1) * H1], pP[:], CP)

            # ---------------- conv phases
            def conv(ci, P_sb, bounce, transposed_scatter, emit_e, e2_sb):
                w1e = cs[f"c{ci}_w1e"]
                b1 = cs[f"c{ci}_b1"]
                w2a = cs[f"c{ci}_w2a"]
                win_psum = {}
                with (
                    tc.tile_pool(name=f"cv{ci}", bufs=3) as sp,
                    tc.tile_pool(name=f"cv{ci}_psA", bufs=2, space="PSUM") as psA,
                    tc.tile_pool(name=f"cv{ci}_psB", bufs=1, space="PSUM") as psB,
                    tc.tile_pool(name=f"cv{ci}_psM", bufs=2, space="PSUM") as psM,
                    tc.tile_pool(name=f"cv{ci}_psW", bufs=2, space="PSUM") as psW,
                    tc.tile_pool(name=f"cv{ci}_psT", bufs=1, space="PSUM") as psT,
                ):
                    for t in range(nt):
                        if e2_sb is None:
                            ef = sp.tile([IN_EDGE, TS], BF16, tag="ef")
                            nc.sync.dma_start(out=ef[:],
                                              in_=inp["efT"][:, t * TS:(t + 1) * TS])
                            # e MLP
                            phe = psA.tile([H1, TS], F32, tag="psA")
                            nc.tensor.matmul(out=phe[:], lhsT=cs["ew1"][:],
                                             rhs=ef[:], start=True, stop=True)
                            he = sp.tile([H1A, TS], BF16, tag="he")
                            nc.scalar.activation(he[:H1, :], phe[:], LR,
                                                 bias=cs["eb1"][:], alpha=SLOPE)
                            nc.gpsimd.memset(he[H1:H1A, :], 1.0)
                            pe = psB.tile([H, TS], F32, tag="psB")
                            nc.tensor.matmul(out=pe[:], lhsT=cs["ew2a"][:],
                                             rhs=he[:], start=True, stop=True)
                            eT = sp.tile([H, TS], BF16, tag="eT")
                            nc.scalar.activation(eT[:], pe[:], LR, alpha=SLOPE)
                            if emit_e:
                                nc.sync.dma_start(out=eT_out[:, t * TS:(t + 1) * TS],
                                                  in_=eT[:])
                            eU = eT
                        else:
                            eU = e2_sb[:, t * TS:(t + 1) * TS]
                        # selection matrices for this tile's chunks:
                        # sels[e, m] = (dstoff[e] == m) built on DVE;
                        # selg = sels^T via PE transpose (for the gather).
                        sel_s = []
                        selg_t = sp.tile([128, TS], BF16, tag="selg", bufs=2)
                        for j in range(per_tile):
                            ch = t * per_tile + j
                            sels = sp.tile([128, 128], BF16, tag="sels",
                                           bufs=2 * per_tile)
                            nc.vector.tensor_tensor(
                                out=sels[:],
                                in0=cs["dstB"][:, ch:ch + 1].to_broadcast([128, 128]),
                                in1=cs["iotaFb"][:],
                                op=mybir.AluOpType.is_equal)
                            pT = psT.tile([128, 128], BF16, tag="psT")
                            nc.tensor.transpose(out=pT[:], in_=sels[:],
                                                identity=cs["idnb"][:])
                            nc.vector.tensor_copy(
                                out=selg_t[:, j * 128:(j + 1) * 128], in_=pT[:])
                            sel_s.append(sels)
                        # msg L1: e-part writes whole tile (incl ones row), then
                        # the per-chunk gather of P[dst] accumulates on top.
                        ph1 = psA.tile([H1, TS], F32, tag="psA")
                        nc.tensor.matmul(out=ph1[:], lhsT=w1e[:],
                                         rhs=eU[:], start=True, stop=False)
                        j = 0
                        while j < per_tile:
                            w = int(win_of_chunk[t * per_tile + j])
                            je = j
                            while (je + 1 < per_tile and
                                   int(win_of_chunk[t * per_tile + je + 1]) == w):
                                je += 1
                            nc.tensor.matmul(
                                out=ph1[:, j * 128:(je + 1) * 128],
                                lhsT=P_sb[:, w * H1:(w + 1) * H1],
                                rhs=selg_t[:, j * 128:(je + 1) * 128],
                                start=False, stop=(je == per_tile - 1))
                            j = je + 1
                        h1 = sp.tile([H1A, TS], BF16, tag="h1")
                        nc.scalar.activation(h1[:H1, :], ph1[:], LR, bias=b1[:],
                                             alpha=SLOPE)
                        nc.gpsimd.memset(h1[H1:H1A, :], 1.0)
                        # per chunk: msg L2 for every chunk first (keeps PE
                        # fed while ACT drains), then the scatters.
                        msgs = []
                        for j in range(per_tile):
                            pmsg = psM.tile([128, 128], F32, tag="psM",
                                            name=f"pmsg{ci}_{t}_{j}")
                            nc.tensor.matmul(out=pmsg[:],
                                             lhsT=h1[:, j * 128:(j + 1) * 128],
                                             rhs=w2a[:], start=True, stop=True)
                            msg = sp.tile([128, 128], BF16, tag="msg",
                                          bufs=2 * per_tile)
                            nc.scalar.activation(msg[:], pmsg[:],
                                                 LR, alpha=SLOPE)
                            msgs.append(msg)
                        for j in range(per_tile):
                            ch = t * per_tile + j
                            w = int(win_of_chunk[ch])
                            msg = msgs[j]
                            sels = sel_s[j]
                            first = ch == int(chunk0[w])
                            last = ch == int(chunk0[w]) + int(C[w]) - 1
                            if first:
                                win_psum[w] = psW.tile([128, 128], F32, tag="psW",
                                                       name=f"win{ci}_{w}")
                            pw = win_psum[w]
                            if transposed_scatter:
                                nc.tensor.matmul(out=pw[:], lhsT=msg[:], rhs=sels[:],
                                                 start=first, stop=last)
                            else:
                                nc.tensor.matmul(out=pw[:], lhsT=sels[:], rhs=msg[:],
                                                 start=first, stop=last)
                            if last:
                                st = sp.tile([128, 128], F32, tag="st")
                                nc.vector.tensor_copy(out=st[:], in_=pw[:])
                                if transposed_scatter:
                                    nc.sync.dma_start(
                                        out=bounce[:, w * WIN:(w + 1) * WIN],
                                        in_=st[:])
                                else:
                                    nc.sync.dma_start(
                                        out=bounce[w * WIN:(w + 1) * WIN, :],
                                        in_=st[:])
                                del win_psum[w]

            conv(1, P1_sb, n1t_in, transposed_scatter=True, emit_e=True,
                 e2_sb=None)

            # AllReduce n1 (transposed layout). The e2 precompute pass below
            # is independent of the collective and fills the AR latency.
            nc.gpsimd.collective_compute(
                "AllReduce", mybir.AluOpType.add,
                replica_groups=[core_ids],
                ins=[n1t_in[:].opt()], outs=[n1t_ar[:].opt()])

            e2T_sb = pp.tile([128, ep], BF16, tag="e2T_sb", name="e2T_sb")
            with (
                tc.tile_pool(name="e2p", bufs=3) as sp,
                tc.tile_pool(name="e2p_psA", bufs=2, space="PSUM") as psA,
                tc.tile_pool(name="e2p_psB", bufs=2, space="PSUM") as psB,
            ):
                for t in range(nt):
                    ef = sp.tile([IN_EDGE, TS], BF16, tag="ef")
                    nc.sync.dma_start(out=ef[:],
                                      in_=inp["efT"][:, t * TS:(t + 1) * TS])
                    phe = psA.tile([H1, TS], F32, tag="psA")
                    nc.tensor.matmul(out=phe[:], lhsT=cs["ew1"][:],
                                     rhs=ef[:], start=True, stop=True)
                    he = sp.tile([H1A, TS], BF16, tag="he")
                    nc.scalar.activation(he[:H1, :], phe[:], LR,
                                         bias=cs["eb1"][:], alpha=SLOPE)
                    nc.gpsimd.memset(he[H1:H1A, :], 1.0)
                    pe = psB.tile([H, TS], F32, tag="psB")
                    nc.tensor.matmul(out=pe[:], lhsT=cs["ew2a"][:],
                                     rhs=he[:], start=True, stop=True)
                    eT = sp.tile([H, TS], BF16, tag="eT")
                    nc.scalar.activation(eT[:], pe[:], LR, alpha=SLOPE)
                    ph2 = psA.tile([H1, TS], F32, tag="psA")
                    nc.tensor.matmul(out=ph2[:], lhsT=cs["e2w1"][:],
                                     rhs=eT[:], start=True, stop=True)
                    h2 = sp.tile([H1A, TS], BF16, tag="he")
                    nc.scalar.activation(h2[:H1, :], ph2[:], LR,
                                         bias=cs["e2b1"][:], alpha=SLOPE)
                    nc.gpsimd.memset(h2[H1:H1A, :], 1.0)
                    pe2 = psB.tile([H, TS], F32, tag="psB")
                    nc.tensor.matmul(out=pe2[:], lhsT=cs["e2w2a"][:],
                                     rhs=h2[:], start=True, stop=True)
                    nc.scalar.activation(e2T_sb[:, t * TS:(t + 1) * TS],
                                         pe2[:], LR, alpha=SLOPE)

            nc.gpsimd.dma_start(out=n1T_sb[:], in_=n1t_ar[:])  # f32 -> bf16 cast
            with tc.tile_pool(name="p2_ps", bufs=2, space="PSUM") as ps:
                for w in range(nw):
                    pP = ps.tile([128, H1], F32, tag="pP2")
                    nc.tensor.matmul(out=pP[:],
                                     lhsT=n1T_sb[:, w * WIN:(w + 1) * WIN],
                                     rhs=cs["c2_w1n"][:], start=True, stop=True)
                    nc.scalar.activation(P2_sb[:, w * H1:(w + 1) * H1], pP[:], CP)

            conv(2, P2_sb, n2_in, transposed_scatter=False, emit_e=False,
                 e2_sb=e2T_sb)

            # ReduceScatter n2 (natural layout) -> graph head shard
            nc.gpsimd.collective_compute(
                "ReduceScatter", mybir.AluOpType.add,
                replica_groups=[core_ids],
                ins=[n2_in[:].opt()], outs=[n2_rs[:].opt()])

            # ---------------- graph head
            with (
                tc.tile_pool(name="gh", bufs=4) as sp,
                tc.tile_pool(name="gh_ps", bufs=2, space="PSUM") as ps,
                tc.tile_pool(name="gh_acc", bufs=1, space="PSUM") as psacc,
            ):
                x_sb = pp.tile([128, rs_rows], BF16, name="xT", tag="xT")
                for b0 in range(0, rs_rows, 128):
                    L = min(128, rs_rows - b0)
                    ld = sp.tile([128, 128], F32, tag="ld")
                    nc.sync.dma_start(out=ld[:L, :], in_=n2_rs[b0:b0 + L, :])
                    pt = ps.tile([128, 128], F32, tag="pt")
                    nc.tensor.transpose(out=pt[:, :L], in_=ld[:L, :],
                                        identity=cs["idn"][:L, :L])
                    nc.scalar.activation(x_sb[:, b0:b0 + L], pt[:, :L], CP)
                nmm = rs_rows  # one matmul per 128-element x chunk (= one node)
                GB = 16        # graph chunks per DMA batch
                pg = psacc.tile([1, H], F32, tag="pg")
                for g in range(0, nmm, GB):
                    gn = min(GB, nmm - g)
                    gw_t = sp.tile([128, GB * H], BF16, tag="gw")
                    nc.sync.dma_start(out=gw_t[:, :gn * H],
                                      in_=inp["gws"][:, g * H:(g + gn) * H])
                    for j in range(gn):
                        i = g + j
                        nc.tensor.matmul(out=pg[:], lhsT=x_sb[:, i:i + 1],
                                         rhs=gw_t[:, j * H:(j + 1) * H],
                                         start=(i == 0), stop=(i == nmm - 1))
                go = pp.tile([1, H], F32, name="go", tag="go")
                nc.scalar.activation(go[:], pg[:], CP)
                nc.sync.dma_start(out=gpart[:], in_=go[:])

    nc.finalize()
    return nc


# ---------------------------------------------------------------- entry point

_CACHE = {}


def _run(inputs, trace=False):
    inputs = {k: np.asarray(v) for k, v in inputs.items()}
    shared, per_core, meta = host_arrays(inputs, N, E, NCORES)
    key = hash(inputs["edge_index"].tobytes())
    if key not in _CACHE:
        _CACHE[key] = build_program(meta, NCORES)
    nc = _CACHE[key]
    in_maps = []
    for c in range(NCORES):
        m = dict(shared)
        m.update({k: v for k, v in per_core[c].items() if k != "slot_global"})
        in_maps.append(m)
    res = run_bass_kernel_spmd(nc, in_maps, list(range(NCORES)), trace=trace)

    # host-side unshard
    e_full = np.empty((E, H), np.float32)
    gsum = np.zeros(H, np.float32)
    for c in range(NCORES):
        out = res.results[c]
        sg = per_core[c]["slot_global"]
        mvalid = sg >= 0
        e_full[sg[mvalid]] = out["eT_out"].astype(np.float32).T[mvalid]
        gsum += out["gpart"][0]
    graph = gsum + inputs["graph_b"]
    graph = np.where(graph >= 0, graph, SLOPE * graph).astype(np.float32)
    return (graph, e_full), res


def kernel(**inputs):
    out, _ = _run(inputs, trace=False)
    return out


def kernel_profiled(**inputs):
    out, res = _run(inputs, trace=True)
    return out, res.exec_time_ns


# revision 23
# speedup vs baseline: 1.2425x; 1.0536x over previous
"""Trainium2 Bass kernel for nn_Encoder (GNN message passing), 8-core SPMD.

Strategy (edge-parallel):
  - Shard E edges across 8 cores; each core sorts its edges by dst node and
    processes them in 128-node "windows" so scatter-add becomes PSUM-matmul
    accumulation against on-device one-hot selection matrices.
  - Gather n[dst] is fused into the message MLP layer 1 by precomputing
    P = n @ w1_node per node; the gather matmul accumulates P[dst] directly
    into the L1 PSUM tile.
  - conv1 node sums accumulate transposed ([H, node]) and are AllReduced;
    conv2 sums accumulate natural ([node, H]) and are ReduceScattered to
    feed the input-dim-sharded graph head (graph_w in bf16, streamed).
  - e / e2 / node MLPs run in float32r; conv machinery runs in bf16 with
    fp32 PSUM accumulation.
"""

import math
import numpy as np
import ml_dtypes

import concourse.bacc as bacc
import concourse.bass as bass
import concourse.mybir as mybir
import concourse.tile as tile
from concourse.bass_utils import run_bass_kernel_spmd

F32 = mybir.dt.float32
F32R = mybir.dt.float32r
BF16 = mybir.dt.bfloat16

# problem constants (full size)
N = 10000
E = 320000
H = 128
NCORES = 8
IN_NODE = 4
IN_EDGE = 3
SLOPE = 0.1
WIN = 128          # nodes per scatter window
TS = 512           # edge slots per streaming tile
H1 = 64            # hidden dim of the 2-layer MLPs
H1A = H1 + 1       # augmented with ones-row for bias folding


# ---------------------------------------------------------------- host prep

def build_schedule(dst, n_nodes, n_edges, ncores):
    """Chunk schedule identical across cores, with edges dealt round-robin
    per destination window so per-(core,window) counts are balanced.

    Returns (C, win_of_chunk, chunk0_of_win, EP, slots) where slots[c] is an
    int64 array [EP] holding the GLOBAL edge index for each slot or -1.
    """
    nw = math.ceil(n_nodes / WIN)
    order = np.argsort(dst, kind="stable")       # all edges by dst
    wins = dst[order] // WIN
    cnt_all = np.bincount(wins, minlength=nw)
    # per-core count for window w: split cnt_all[w] as evenly as possible
    percore = -(-cnt_all[None, :] // ncores)     # ceil
    C = np.maximum(1, np.ceil(percore.max(axis=0) / 128).astype(np.int64))
    per_tile = TS // 128
    pad = (-int(C.sum())) % per_tile
    C[nw - 1] += pad
    nch = int(C.sum())
    ep = nch * 128
    chunk0 = np.zeros(nw, np.int64)
    chunk0[1:] = np.cumsum(C)[:-1]
    win_of_chunk = np.repeat(np.arange(nw), C)
    slots = [np.full(ep, -1, np.int64) for _ in range(ncores)]
    w0 = np.zeros(nw + 1, np.int64)
    w0[1:] = np.cumsum(cnt_all)
    for w in range(nw):
        ge = order[w0[w]:w0[w + 1]]              # global edges of window w
        base = 128 * int(chunk0[w])
        for c in range(ncores):
            mine = ge[c::ncores]
            slots[c][base:base + len(mine)] = mine
    return C, win_of_chunk, chunk0, ep, slots


def host_arrays(inputs, n_nodes, n_edges, ncores):
    """Build all per-core and shared device input arrays."""
    e_loc = n_edges // ncores
    nw = math.ceil(n_nodes / WIN)
    np_pad = nw * WIN
    dst = np.asarray(inputs["edge_index"][1])
    C, win_of_chunk, chunk0, ep, slots = build_schedule(dst, n_nodes, n_edges, ncores)
    nch = ep // 128
    ga = math.ceil(nch / 128)

    bf = lambda x: np.ascontiguousarray(x).astype(ml_dtypes.bfloat16)
    f32 = lambda x: np.ascontiguousarray(x, np.float32)

    shared = {}
    # node MLP (float32r path, stored f32)
    shared["nw1"] = bf(inputs["node_w1"])                 # [4,64]
    shared["nb1"] = f32(np.asarray(inputs["node_b1"])[:, None])  # [64,1]
    shared["nw2"] = bf(inputs["node_w2"])                  # [64,128]
    shared["nb2"] = f32(np.asarray(inputs["node_b2"])[:, None])  # [128,1]
    # edge MLP
    shared["ew1"] = bf(inputs["edge_w1"])                 # [3,64]
    shared["eb1"] = f32(np.asarray(inputs["edge_b1"])[:, None])
    shared["ew2"] = bf(inputs["edge_w2"])                  # [64,128]
    shared["eb2"] = f32(np.asarray(inputs["edge_b2"])[:, None])
    # e2 MLP (last=True edge transform)
    shared["e2w1"] = bf(inputs["g2e_w1"])                 # [128,64]
    shared["e2b1"] = f32(np.asarray(inputs["g2e_b1"])[:, None])
    shared["e2w2"] = bf(inputs["g2e_w2"])                  # [64,128]
    shared["e2b2"] = f32(np.asarray(inputs["g2e_b2"])[:, None])
    # conv weights (bf16 machinery)
    for i, pre in ((1, "g1"), (2, "g2")):
        w1 = np.asarray(inputs[f"{pre}_w1"])  # [2H, 64]
        shared[f"c{i}_w1n"] = bf(w1[:H])                      # [128,64]
        shared[f"c{i}_w1e"] = bf(w1[H:])                     # [128,64] f32r
        shared[f"c{i}_b1"] = f32(np.asarray(inputs[f"{pre}_b1"])[:, None])  # [64,1]
        shared[f"c{i}_w2a"] = bf(np.concatenate(
            [inputs[f"{pre}_w2"], inputs[f"{pre}_b2"][None, :]], axis=0))  # [65,128]
    # consts
    shared["iotaFb"] = bf(np.tile(np.arange(128, dtype=np.float32), (128, 1)))
    shared["idn"] = f32(np.eye(128, dtype=np.float32))
    shared["idnb"] = bf(np.eye(128, dtype=np.float32))
    # node features, padded + transposed
    nf = np.zeros((np_pad, IN_NODE), np.float32)
    nf[:n_nodes] = inputs["node_feat"]
    shared["nfT"] = bf(nf.T)  # [4, NP]

    # per-core arrays
    per_core = []
    ef = np.asarray(inputs["edge_feat"])
    rs_rows = np_pad // ncores
    gw = np.zeros((np_pad * H, H), np.float32)
    gw[:n_nodes * H] = inputs["graph_w"]
    for c in range(ncores):
        slot = slots[c]
        valid = slot >= 0
        eft = np.zeros((IN_EDGE, ep), np.float32)
        eft[:, valid] = ef[slot[valid]].T
        eft = bf(eft)
        dstoff = np.full(ep, -1.0, np.float32)
        dstoff[valid] = (dst[slot[valid]] - WIN * win_of_chunk[
            np.arange(ep) // 128][valid]).astype(np.float32)
        dstB = bf(dstoff.reshape(nch, 128).T)          # [128, NCH] bf16
        # graph head weight shard, shuffled so rhs tiles DMA contiguously:
        # gws[k, 128*i + h] = gw_shard[128*i + k, h]
        gshard = gw[c * rs_rows * H:(c + 1) * rs_rows * H]   # [rs_rows*H, H]
        nchunk = gshard.shape[0] // 128
        gws = bf(gshard.reshape(nchunk, 128, H).transpose(1, 0, 2).reshape(128, -1))
        per_core.append({
            "efT": eft, "dstB": dstB, "gws": gws,
            "slot_global": slot,
        })
    meta = dict(C=C, win_of_chunk=win_of_chunk, chunk0=chunk0, ep=ep,
                nch=nch, ga=ga, nw=nw, np_pad=np_pad, e_loc=e_loc,
                rs_rows=rs_rows)
    return shared, per_core, meta


# ---------------------------------------------------------------- device prog

def build_program(meta, n_cores):
    nw, np_pad, ep, nch, ga = (meta["nw"], meta["np_pad"], meta["ep"],
                               meta["nch"], meta["ga"])
    C, win_of_chunk, chunk0 = meta["C"], meta["win_of_chunk"], meta["chunk0"]
    rs_rows = meta["rs_rows"]
    nt = ep // TS
    per_tile = TS // 128

    nc = bacc.Bacc(num_devices=n_cores)
    core_ids = list(range(n_cores))

    # ---- I/O declarations
    inp = {}
    def din(name, shape, dtype=F32):
        inp[name] = nc.declare_dram_parameter(name, list(shape), dtype, isOutput=False)
        return inp[name]

    for nm, sh in [
        ("nb1", (H1, 1)), ("eb1", (H1, 1)), ("e2b1", (H1, 1)),
        ("nb2", (H, 1)), ("eb2", (H, 1)), ("e2b2", (H, 1)),
        ("c1_b1", (H1, 1)), ("c2_b1", (H1, 1)),
        ("idn", (128, 128)),
    ]:
        din(nm, sh)
    din("iotaFb", (128, 128), BF16)
    din("dstB", (128, nch), BF16)
    for nm, sh in [("nw1", (IN_NODE, H1)), ("nw2", (H1, H)),
                   ("ew1", (IN_EDGE, H1)), ("ew2", (H1, H)),
                   ("e2w1", (H, H1)), ("e2w2", (H1, H)),
                   ("c1_w1e", (H, H1)), ("c2_w1e", (H, H1)),
                   ("c1_w1n", (H, H1)), ("c1_w2a", (H1A, H)),
                   ("c2_w1n", (H, H1)), ("c2_w2a", (H1A, H)),
                   ("idnb", (128, 128)), ("nfT", (IN_NODE, np_pad)),
                   ("efT", (IN_EDGE, ep)),
                   ("gws", (128, rs_rows * H))]:
        din(nm, sh, BF16)

    eT_out = nc.declare_dram_parameter("eT_out", [128, ep], BF16, isOutput=True)
    gpart = nc.declare_dram_parameter("gpart", [1, H], F32, isOutput=True)

    LR = mybir.ActivationFunctionType.Prelu
    CP = mybir.ActivationFunctionType.Copy

    with tile.TileContext(nc, num_cores=n_cores) as tc:
        with (
            tc.tile_pool(name="const", bufs=1) as cp,
            tc.tile_pool(name="persist", bufs=1) as pp,
            tc.tile_pool(name="dram", bufs=1, space="DRAM") as dp,
        ):
            # constants into SBUF
            cs = {}
            for nm in ["nb1", "eb1", "e2b1", "nb2", "eb2", "e2b2",
                       "c1_b1", "c2_b1", "idn"]:
                t = cp.tile(list(inp[nm].shape), F32, name=f"cs_{nm}", tag=f"cs_{nm}")
                nc.sync.dma_start(out=t[:], in_=inp[nm][:])
                cs[nm] = t
            for nm in ["nw1", "nw2", "ew1", "ew2", "e2w1", "e2w2",
                       "c1_w1e", "c2_w1e", "iotaFb", "dstB",
                       "c1_w1n", "c1_w2a", "c2_w1n", "c2_w2a", "idnb"]:
                t = cp.tile(list(inp[nm].shape), BF16, name=f"cs_{nm}", tag=f"cs_{nm}")
                nc.sync.dma_start(out=t[:], in_=inp[nm][:])
                cs[nm] = t

            # persistent activations
            P1_sb = pp.tile([128, nw * H1], BF16, name="P1", tag="P1")
            P2_sb = pp.tile([128, nw * H1], BF16, name="P2", tag="P2")
            n1T_sb = pp.tile([128, np_pad], BF16, name="n1T", tag="n1T")

            # collective bounce buffers
            n1t_in = dp.tile([128, np_pad], F32, name="n1t_in", tag="n1t_in")
            n1t_ar = dp.tile([128, np_pad], F32, name="n1t_ar", tag="n1t_ar")
            n2_in = dp.tile([np_pad, 128], F32, name="n2_in", tag="n2_in")
            n2_rs = dp.tile([rs_rows, 128], F32, name="n2_rs", tag="n2_rs")

            # ---------------- phase: node MLP + P1
            with (
                tc.tile_pool(name="nmlp", bufs=3) as sp,
                tc.tile_pool(name="nmlp_ps", bufs=2, space="PSUM") as ps,
            ):
                for g0 in range(0, nw, 4):          # 4 windows = 512 nodes/tile
                    wn = min(4, nw - g0)
                    L = wn * WIN
                    nf = sp.tile([IN_NODE, L], BF16, tag="nf")
                    nc.sync.dma_start(out=nf[:], in_=inp["nfT"][:, g0 * WIN:g0 * WIN + L])
                    ph = ps.tile([H1, TS], F32, tag="ph")
                    nc.tensor.matmul(out=ph[:, :L], lhsT=cs["nw1"][:],
                                     rhs=nf[:], start=True, stop=True)
                    hh = sp.tile([H1, L], BF16, tag="hh")
                    nc.scalar.activation(hh[:], ph[:, :L], LR, bias=cs["nb1"][:],
                                         alpha=SLOPE)
                    pn = ps.tile([H, TS], F32, tag="pn")
                    nc.tensor.matmul(out=pn[:, :L], lhsT=cs["nw2"][:],
                                     rhs=hh[:], start=True, stop=True)
                    nT = sp.tile([H, L], BF16, tag="nT")
                    nc.scalar.activation(nT[:], pn[:, :L], LR, bias=cs["nb2"][:],
                                         alpha=SLOPE)
                    for k in range(wn):
                        w = g0 + k
                        pP = ps.tile([128, H1], F32, tag="pP")
                        nc.tensor.matmul(out=pP[:], lhsT=nT[:, k * WIN:(k + 1) * WIN],
                                         rhs=cs["c1_w1n"][:], start=True, stop=True)
                        nc.scalar.activation(P1_sb[:, w * H1:(w + 1) * H1], pP[:], CP)

            # ---------------- conv phases
            def conv(ci, P_sb, bounce, transposed_scatter, emit_e, e2_sb):
                w1e = cs[f"c{ci}_w1e"]
                b1 = cs[f"c{ci}_b1"]
                w2a = cs[f"c{ci}_w2a"]
                win_psum = {}
                with (
                    tc.tile_pool(name=f"cv{ci}", bufs=3) as sp,
                    tc.tile_pool(name=f"cv{ci}_psA", bufs=2, space="PSUM") as psA,
                    tc.tile_pool(name=f"cv{ci}_psB", bufs=1, space="PSUM") as psB,
                    tc.tile_pool(name=f"cv{ci}_psM", bufs=2, space="PSUM") as psM,
                    tc.tile_pool(name=f"cv{ci}_psW", bufs=2, space="PSUM") as psW,
                    tc.tile_pool(name=f"cv{ci}_psT", bufs=1, space="PSUM") as psT,
                ):
                    for t in range(nt):
                        if e2_sb is None:
                            ef = sp.tile([IN_EDGE, TS], BF16, tag="ef")
                            nc.sync.dma_start(out=ef[:],
                                              in_=inp["efT"][:, t * TS:(t + 1) * TS])
                            # e MLP
                            phe = psA.tile([H1, TS], F32, tag="psA")
                            nc.tensor.matmul(out=phe[:], lhsT=cs["ew1"][:],
                                             rhs=ef[:], start=True, stop=True)
                            he = sp.tile([H1, TS], BF16, tag="he")
                            nc.scalar.activation(he[:], phe[:], LR,
                                                 bias=cs["eb1"][:], alpha=SLOPE)
                            pe = psB.tile([H, TS], F32, tag="psB")
                            nc.tensor.matmul(out=pe[:], lhsT=cs["ew2"][:],
                                             rhs=he[:], start=True, stop=True)
                            eT = sp.tile([H, TS], BF16, tag="eT")
                            nc.scalar.activation(eT[:], pe[:], LR,
                                                 bias=cs["eb2"][:], alpha=SLOPE)
                            if emit_e:
                                nc.sync.dma_start(out=eT_out[:, t * TS:(t + 1) * TS],
                                                  in_=eT[:])
                            eU = eT
                        else:
                            eU = e2_sb[:, t * TS:(t + 1) * TS]
                        # selection matrices for this tile's chunks:
                        # sels[e, m] = (dstoff[e] == m) built on DVE;
                        # selg = sels^T via PE transpose (for the gather).
                        sel_s = []
                        selg_t = sp.tile([128, TS], BF16, tag="selg", bufs=2)
                        for j in range(per_tile):
                            ch = t * per_tile + j
                            sels = sp.tile([128, 128], BF16, tag="sels",
                                           bufs=2 * per_tile)
                            nc.vector.tensor_tensor(
                                out=sels[:],
                                in0=cs["dstB"][:, ch:ch + 1].to_broadcast([128, 128]),
                                in1=cs["iotaFb"][:],
                                op=mybir.AluOpType.is_equal)
                            pT = psT.tile([128, 128], BF16, tag="psT")
                            nc.tensor.transpose(out=pT[:], in_=sels[:],
                                                identity=cs["idnb"][:])
                            nc.vector.tensor_copy(
                                out=selg_t[:, j * 128:(j + 1) * 128], in_=pT[:])
                            sel_s.append(sels)
                        # msg L1: e-part writes whole tile (incl ones row), then
                        # the per-chunk gather of P[dst] accumulates on top.
                        ph1 = psA.tile([H1, TS], F32, tag="psA")
                        nc.tensor.matmul(out=ph1[:], lhsT=w1e[:],
                                         rhs=eU[:], start=True, stop=False)
                        j = 0
                        while j < per_tile:
                            w = int(win_of_chunk[t * per_tile + j])
                            je = j
                            while (je + 1 < per_tile and
                                   int(win_of_chunk[t * per_tile + je + 1]) == w):
                                je += 1
                            nc.tensor.matmul(
                                out=ph1[:, j * 128:(je + 1) * 128],
                                lhsT=P_sb[:, w * H1:(w + 1) * H1],
                                rhs=selg_t[:, j * 128:(je + 1) * 128],
                                start=False, stop=(je == per_tile - 1))
                            j = je + 1
                        h1 = sp.tile([H1A, TS], BF16, tag="h1")
                        nc.scalar.activation(h1[:H1, :], ph1[:], LR, bias=b1[:],
                                             alpha=SLOPE)
                        nc.gpsimd.memset(h1[H1:H1A, :], 1.0)
                        # per chunk: msg L2 for every chunk first (keeps PE
                        # fed while ACT drains), then the scatters.
                        msgs = []
                        for j in range(per_tile):
                            pmsg = psM.tile([128, 128], F32, tag="psM",
                                            name=f"pmsg{ci}_{t}_{j}")
                            nc.tensor.matmul(out=pmsg[:],
                                             lhsT=h1[:, j * 128:(j + 1) * 128],
                                             rhs=w2a[:], start=True, stop=True)
                            msg = sp.tile([128, 128], BF16, tag="msg",
                                          bufs=2 * per_tile)
                            nc.scalar.activation(msg[:], pmsg[:],
                                                 LR, alpha=SLOPE)
                            msgs.append(msg)
                        for j in range(per_tile):
                            ch = t * per_tile + j
                            w = int(win_of_chunk[ch])
                            msg = msgs[j]
                            sels = sel_s[j]
                            first = ch == int(chunk0[w])
                            last = ch == int(chunk0[w]) + int(C[w]) - 1
                            if first:
                                win_psum[w] = psW.tile([128, 128], F32, tag="psW",
                                                       name=f"win{ci}_{w}")
                            pw = win_psum[w]
                            if transposed_scatter:
                                nc.tensor.matmul(out=pw[:], lhsT=msg[:], rhs=sels[:],
                                                 start=first, stop=last)
                            else:
                                nc.tensor.matmul(out=pw[:], lhsT=sels[:], rhs=msg[:],
                                                 start=first, stop=last)
                            if last:
                                st = sp.tile([128, 128], F32, tag="st")
                                nc.vector.tensor_copy(out=st[:], in_=pw[:])
                                if transposed_scatter:
                                    nc.sync.dma_start(
                                        out=bounce[:, w * WIN:(w + 1) * WIN],
                                        in_=st[:])
                                else:
                                    nc.sync.dma_start(
                                        out=bounce[w * WIN:(w + 1) * WIN, :],
                                        in_=st[:])
                                del win_psum[w]

            conv(1, P1_sb, n1t_in, transposed_scatter=True, emit_e=True,
                 e2_sb=None)

            # AllReduce n1 (transposed layout). The e2 precompute pass below
            # is independent of the collective and fills the AR latency.
            nc.gpsimd.collective_compute(
                "AllReduce", mybir.AluOpType.add,
                replica_groups=[core_ids],
                ins=[n1t_in[:].opt()], outs=[n1t_ar[:].opt()])

            e2T_sb = pp.tile([128, ep], BF16, tag="e2T_sb", name="e2T_sb")
            with (
                tc.tile_pool(name="e2p", bufs=3) as sp,
                tc.tile_pool(name="e2p_psA", bufs=2, space="PSUM") as psA,
                tc.tile_pool(name="e2p_psB", bufs=2, space="PSUM") as psB,
            ):
                for t in range(nt):
                    ef = sp.tile([IN_EDGE, TS], BF16, tag="ef")
                    nc.sync.dma_start(out=ef[:],
                                      in_=inp["efT"][:, t * TS:(t + 1) * TS])
                    phe = psA.tile([H1, TS], F32, tag="psA")
                    nc.tensor.matmul(out=phe[:], lhsT=cs["ew1"][:],
                                     rhs=ef[:], start=True, stop=True)
                    he = sp.tile([H1, TS], BF16, tag="he")
                    nc.scalar.activation(he[:], phe[:], LR,
                                         bias=cs["eb1"][:], alpha=SLOPE)
                    pe = psB.tile([H, TS], F32, tag="psB")
                    nc.tensor.matmul(out=pe[:], lhsT=cs["ew2"][:],
                                     rhs=he[:], start=True, stop=True)
                    eT = sp.tile([H, TS], BF16, tag="eT")
                    nc.scalar.activation(eT[:], pe[:], LR,
                                         bias=cs["eb2"][:], alpha=SLOPE)
                    ph2 = psA.tile([H1, TS], F32, tag="psA")
                    nc.tensor.matmul(out=ph2[:], lhsT=cs["e2w1"][:],
                                     rhs=eT[:], start=True, stop=True)
                    h2 = sp.tile([H1, TS], BF16, tag="he")
                    nc.scalar.activation(h2[:], ph2[:], LR,
                                         bias=cs["e2b1"][:], alpha=SLOPE)
                    pe2 = psB.tile([H, TS], F32, tag="psB")
                    nc.tensor.matmul(out=pe2[:], lhsT=cs["e2w2"][:],
                                     rhs=h2[:], start=True, stop=True)
                    nc.scalar.activation(e2T_sb[:, t * TS:(t + 1) * TS],
                                         pe2[:], LR, bias=cs["e2b2"][:],
                                         alpha=SLOPE)

            nc.gpsimd.dma_start(out=n1T_sb[:], in_=n1t_ar[:])  # f32 -> bf16 cast
            with tc.tile_pool(name="p2_ps", bufs=2, space="PSUM") as ps:
                for w in range(nw):
                    pP = ps.tile([128, H1], F32, tag="pP2")
                    nc.tensor.matmul(out=pP[:],
                                     lhsT=n1T_sb[:, w * WIN:(w + 1) * WIN],
                                     rhs=cs["c2_w1n"][:], start=True, stop=True)
                    nc.scalar.activation(P2_sb[:, w * H1:(w + 1) * H1], pP[:], CP)

            conv(2, P2_sb, n2_in, transposed_scatter=False, emit_e=False,
                 e2_sb=e2T_sb)

            # ReduceScatter n2 (natural layout) -> graph head shard
            nc.gpsimd.collective_compute(
                "ReduceScatter", mybir.AluOpType.add,
                replica_groups=[core_ids],
                ins=[n2_in[:].opt()], outs=[n2_rs[:].opt()])

            # ---------------- graph head
            with (
                tc.tile_pool(name="gh", bufs=4) as sp,
                tc.tile_pool(name="gh_ps", bufs=2, space="PSUM") as ps,
                tc.tile_pool(name="gh_acc", bufs=1, space="PSUM") as psacc,
            ):
                x_sb = pp.tile([128, rs_rows], BF16, name="xT", tag="xT")
                for b0 in range(0, rs_rows, 128):
                    L = min(128, rs_rows - b0)
                    ld = sp.tile([128, 128], F32, tag="ld")
                    nc.sync.dma_start(out=ld[:L, :], in_=n2_rs[b0:b0 + L, :])
                    pt = ps.tile([128, 128], F32, tag="pt")
                    nc.tensor.transpose(out=pt[:, :L], in_=ld[:L, :],
                                        identity=cs["idn"][:L, :L])
                    nc.scalar.activation(x_sb[:, b0:b0 + L], pt[:, :L], CP)
                nmm = rs_rows  # one matmul per 128-element x chunk (= one node)
                GB = 16        # graph chunks per DMA batch
                pg = psacc.tile([1, H], F32, tag="pg")
                for g in range(0, nmm, GB):
                    gn = min(GB, nmm - g)
                    gw_t = sp.tile([128, GB * H], BF16, tag="gw")
                    nc.sync.dma_start(out=gw_t[:, :gn * H],
                                      in_=inp["gws"][:, g * H:(g + gn) * H])
                    for j in range(gn):
                        i = g + j
                        nc.tensor.matmul(out=pg[:], lhsT=x_sb[:, i:i + 1],
                                         rhs=gw_t[:, j * H:(j + 1) * H],
                                         start=(i == 0), stop=(i == nmm - 1))
                go = pp.tile([1, H], F32, name="go", tag="go")
                nc.scalar.activation(go[:], pg[:], CP)
                nc.sync.dma_start(out=gpart[:], in_=go[:])

    nc.finalize()
    return nc


# ---------------------------------------------------------------- entry point

_CACHE = {}


def _run(inputs, trace=False):
    inputs = {k: np.asarray(v) for k, v in inputs.items()}
    shared, per_core, meta = host_arrays(inputs, N, E, NCORES)
    key = hash(inputs["edge_index"].tobytes())
    if key not in _CACHE:
        _CACHE[key] = build_program(meta, NCORES)
    nc = _CACHE[key]
    in_maps = []
    for c in range(NCORES):
        m = dict(shared)
        m.update({k: v for k, v in per_core[c].items() if k != "slot_global"})
        in_maps.append(m)
    res = run_bass_kernel_spmd(nc, in_maps, list(range(NCORES)), trace=trace)

    # host-side unshard
    e_full = np.empty((E, H), np.float32)
    gsum = np.zeros(H, np.float32)
    for c in range(NCORES):
        out = res.results[c]
        sg = per_core[c]["slot_global"]
        mvalid = sg >= 0
        e_full[sg[mvalid]] = out["eT_out"].astype(np.float32).T[mvalid]
        gsum += out["gpart"][0]
    graph = gsum + inputs["graph_b"]
    graph = np.where(graph >= 0, graph, SLOPE * graph).astype(np.float32)
    return (graph, e_full), res


def kernel(**inputs):
    out, _ = _run(inputs, trace=False)
    return out


def kernel_profiled(**inputs):
    out, res = _run(inputs, trace=True)
    return out, res.exec_time_ns
